# revision 16
# baseline (speedup 1.0000x reference)
"""Trainium2 Bass kernel for a pre-norm look-ahead-window transformer block.

Problem: y = x + FFN(LN2(x + Attn(LN1(x)))) with banded-causal attention
(key j visible to query i iff j <= i + WIN), B=2, T=2048, D=1024, H=16.

Distribution over 8 NeuronCores (one SPMD program, uniform across cores):
  core c: batch b = c//4, rank r = c%4 in the 4-core batch group.
  - residual / out-proj / FFN are token-sharded (512 tokens per core).
  - QKV + attention are head-sharded (4 heads, all 2048 tokens), which keeps
    the banded-attention loop structure identical on every core. LN1 is
    folded into the QKV weights on the host (W' = W*g per column, rank-1
    mean correction via a K=1 matmul, per-token rstd scale in the epilogue);
    LN1 stats for all tokens are computed locally on every core (cheaper
    than a collective's latency floor).
  - ONE 4-rank AllGather moves the attention output O^T (1 MB bf16); all
    FFN1 weights are prefetched into SBUF underneath it. The post-gather
    token slice is selected with a partition-id-driven dynamic DMA offset.

Everything runs in a transposed [feature, token] layout so matmuls contract
over SBUF partitions; the host pre-transposes x / weights and transposes
the per-core [D, 512] output shards back. Matmul inputs are bf16 (full PE
rate); residuals and PSUM accumulation stay fp32.
"""

import functools

import numpy as np

B, T, D, H, HD, WIN, F = 2, 2048, 1024, 16, 64, 10, 4096
P = 128
NCORES = 8
TOK = 512          # tokens per core for the token-sharded phases
DT = D // P        # 8 d-tiles
FT = F // P        # 32 f-tiles
KT = T // P        # 16 key tiles
EM = 2 * P         # 256 e-rows per core per projection (4 heads x 64)
EPS = 1e-5
NEG = -30000.0     # additive mask; exp(NEG) == 0 exactly on ACT


def _emit(nc, tile, mybir):
    import concourse.bass as bass

    f32 = mybir.dt.float32
    f32r = mybir.dt.float32r
    bf16 = mybir.dt.bfloat16
    ALU = mybir.AluOpType
    ACT = mybir.ActivationFunctionType

    # ---- kernel I/O ----
    xTf = nc.dram_tensor("xTf", [D, T], bf16, kind="ExternalInput")
    xTm = nc.dram_tensor("xTm", [D, TOK], f32, kind="ExternalInput")
    wqkvT = nc.dram_tensor("wqkvT", [D, 3 * EM], bf16, kind="ExternalInput")
    uqk = nc.dram_tensor("uqk", [1, 2 * EM], bf16, kind="ExternalInput")
    uv = nc.dram_tensor("uv", [1, EM], bf16, kind="ExternalInput")
    cqk = nc.dram_tensor("cqk", [2 * EM], f32, kind="ExternalInput")
    cv = nc.dram_tensor("cv", [1, EM], f32, kind="ExternalInput")
    outwT = nc.dram_tensor("outwT", [D, D], bf16, kind="ExternalInput")
    w1T = nc.dram_tensor("w1T", [D, F], bf16, kind="ExternalInput")
    w2T = nc.dram_tensor("w2T", [F, D], bf16, kind="ExternalInput")
    outb = nc.dram_tensor("outb", [D], f32, kind="ExternalInput")
    b1 = nc.dram_tensor("b1", [F], f32, kind="ExternalInput")
    b2 = nc.dram_tensor("b2", [D], f32, kind="ExternalInput")
    ln2g = nc.dram_tensor("ln2g", [D], f32, kind="ExternalInput")
    ln2b = nc.dram_tensor("ln2b", [D], f32, kind="ExternalInput")
    maskA = nc.dram_tensor("maskA", [P, P], f32, kind="ExternalInput")
    maskB = nc.dram_tensor("maskB", [P, P], f32, kind="ExternalInput")
    yT = nc.dram_tensor("yT", [D, TOK], f32, kind="ExternalOutput")

    groups = [[0, 1, 2, 3], [4, 5, 6, 7]]

    with tile.TileContext(nc) as tc:
        from contextlib import ExitStack

        with ExitStack() as ctx:
            consts = ctx.enter_context(tc.tile_pool(name="consts", bufs=1))
            epil = ctx.enter_context(tc.tile_pool(name="epil", bufs=4))
            wpool = ctx.enter_context(tc.tile_pool(name="wpool", bufs=6))
            dram = ctx.enter_context(tc.tile_pool(name="dram", bufs=1, space="DRAM"))

            # ---- constants ----
            ones_f = consts.tile([P, 1], f32)
            nc.vector.memset(ones_f, 1.0)
            ones = consts.tile([P, 1], bf16)
            nc.vector.tensor_copy(ones, ones_f)
            ones64r = consts.tile([1, HD], f32r)
            nc.vector.tensor_copy(ones64r, ones_f[0:1, 0:1].to_broadcast((1, HD)))
            mA = consts.tile([P, P], f32)
            nc.sync.dma_start(mA, maskA[:, :])
            mB = consts.tile([P, P], f32)
            nc.sync.dma_start(mB, maskB[:, :])

            def load_pp(dramvec, n):  # [n*P] -> [P, n] per-partition layout
                t = consts.tile([P, n], f32, tag=f"pp_{dramvec.name}")
                nc.sync.dma_start(t, dramvec[:].rearrange("(n p) -> p n", p=P))
                return t

            cqk_t = load_pp(cqk, 4)
            outb_t = load_pp(outb, DT)
            b1_t = load_pp(b1, FT)
            b2_t = load_pp(b2, DT)
            g2t = load_pp(ln2g, DT)
            bt2t = load_pp(ln2b, DT)
            epst = consts.tile([1, 1], f32)
            nc.vector.memset(epst, EPS)
            uqk_s = consts.tile([1, 2 * EM], bf16)
            nc.sync.dma_start(uqk_s, uqk[:, :])
            uv_s = consts.tile([1, EM], bf16)
            nc.sync.dma_start(uv_s, uv[:, :])
            cvb = consts.tile([P, EM], f32)
            nc.gpsimd.dma_start(cvb, cv[:, :].partition_broadcast(P))
            wvres = consts.tile([P, DT, EM], bf16)   # V weights, resident
            nc.sync.dma_start(
                wvres, wqkvT[:, 2 * EM:].rearrange("(dt p) e -> p dt e", p=P))
            wqkres = consts.tile([P, DT, 4, P], bf16)  # Q/K weights, resident
            nc.sync.dma_start(
                wqkres, wqkvT[:, 0:2 * EM]
                .rearrange("(dt p) (e c) -> p dt e c", p=P, e=4))
            owres = consts.tile([P, DT, D], bf16)    # out-proj weights, resident
            nc.sync.dma_start(
                owres, outwT[:, :].rearrange("(dp p) e -> p dp e", p=P))

            cc2_in = dram.tile([EM, T], bf16)
            cc2_out = dram.tile([4 * EM, T], bf16)
            stats_d = dram.tile([2, 4, TOK], f32)    # [0]=rstd, [1]=negm rows

            # q/k/v for my 4 heads over all tokens, then banded attention
            with tc.tile_pool(name="p_qkv", bufs=1) as p_qkv:
                xfull = p_qkv.tile([P, DT, 4, TOK], bf16, tag="xfull")
                nc.sync.dma_start(
                    xfull,
                    xTf[:, :].rearrange("(dt p) (tg t) -> p dt tg t", p=P, t=TOK))
                # qk[:, 0:2, tg, :] = Q^T e-tiles, qk[:, 2:4, tg, :] = K^T
                qk = p_qkv.tile([P, 4, 4, TOK], bf16, tag="qk")
                vb = p_qkv.tile([P, KT, 4, HD + 1], bf16, tag="vb")
                ones_kh = p_qkv.tile([P, KT * 4], f32, tag="ones_kh")
                nc.vector.memset(ones_kh, 1.0)
                nc.vector.tensor_copy(
                    vb[:, :, :, HD],
                    ones_kh.rearrange("p (k h) -> p k h", h=4))

                with tc.tile_pool(name="qkvc", bufs=1) as qkvc:
                    # ---- LN1 stats for ALL tokens, locally ----
                    negm = qkvc.tile([1, 4, TOK], bf16)
                    with (
                        tc.tile_pool(name="st", bufs=2) as st,
                        tc.tile_pool(name="stps", bufs=2, space="PSUM") as stps,
                    ):
                        for tg in range(4):
                            ps_s = stps.tile([1, TOK], f32, tag="ps_s",
                                             name="ps_s")
                            ps_q = stps.tile([1, TOK], f32, tag="ps_q",
                                             name="ps_q")
                            for dt in range(DT):
                                nc.tensor.matmul(ps_s, ones, xfull[:, dt, tg, :],
                                                 start=(dt == 0),
                                                 stop=(dt == DT - 1))
                                sq = st.tile([P, TOK], bf16, tag="sq")
                                nc.vector.tensor_mul(sq, xfull[:, dt, tg, :],
                                                     xfull[:, dt, tg, :])
                                nc.tensor.matmul(ps_q, ones, sq,
                                                 start=(dt == 0),
                                                 stop=(dt == DT - 1))
                            mean = st.tile([1, TOK], f32, tag="mean")
                            nc.vector.tensor_scalar_mul(mean, ps_s, 1.0 / D)
                            var = st.tile([1, TOK], f32, tag="var")
                            nc.vector.tensor_scalar_mul(var, ps_q, 1.0 / D)
                            msq = st.tile([1, TOK], f32, tag="msq")
                            nc.vector.tensor_mul(msq, mean, mean)
                            nc.vector.tensor_sub(var, var, msq)
                            lnv = st.tile([1, TOK], f32, tag="lnv")
                            nc.scalar.activation(out=lnv, in_=var, func=ACT.Ln,
                                                 bias=epst, scale=1.0)
                            rstd1 = st.tile([1, TOK], f32, tag="rstd1")
                            nc.scalar.activation(out=rstd1, in_=lnv,
                                                 func=ACT.Exp, scale=-0.5)
                            nc.sync.dma_start(stats_d[0, tg, :], rstd1)
                            nc.vector.tensor_scalar_mul(negm[:, tg, :], mean,
                                                        -1.0)
                    rT = qkvc.tile([P, KT], f32)          # rstd, token-partition
                    nc.sync.dma_start(
                        rT, stats_d.opt()[0].rearrange("(tt p) -> p tt", p=P))
                    rbc = qkvc.tile([P, 4, TOK], f32)     # rstd bcast over parts
                    nc.gpsimd.dma_start(
                        rbc, stats_d.opt()[0:1].partition_broadcast(P))

                    # ---- QKV matmuls (all weights + x resident in SBUF) ----
                    with (
                        tc.tile_pool(name="qkps", bufs=1, space="PSUM") as qkps,
                        tc.tile_pool(name="vps", bufs=1, space="PSUM") as vps,
                    ):
                        for tg in range(4):
                            ps_qk = [qkps.tile([P, TOK], f32, tag=f"ps{e}",
                                               name=f"ps{e}") for e in range(4)]
                            ps_v = [vps.tile([P, 2, EM], f32, tag=f"psv{s}",
                                             name=f"psv{s}") for s in range(2)]
                            for dt in range(DT):
                                for et in range(4):
                                    nc.tensor.matmul(
                                        ps_qk[et], wqkres[:, dt, et, :],
                                        xfull[:, dt, tg, :],
                                        start=(dt == 0), stop=False)
                                for sub in range(4):
                                    nc.tensor.matmul(
                                        ps_v[sub // 2][:, sub % 2, :],
                                        xfull[:, dt, tg, sub * P:(sub + 1) * P],
                                        wvres[:, dt, :],
                                        start=(dt == 0), stop=False)
                            for et in range(4):
                                nc.tensor.matmul(
                                    ps_qk[et], uqk_s[:, et * P:(et + 1) * P],
                                    negm[:, tg, :], start=False, stop=True)
                                t = epil.tile([P, TOK], f32, tag="ep")
                                nc.vector.tensor_mul(t, ps_qk[et], rbc[:, tg, :])
                                nc.vector.tensor_scalar_add(
                                    qk[:, et, tg, :], t,
                                    scalar1=cqk_t[:, et:et + 1])
                            for sub in range(4):
                                tt = tg * 4 + sub
                                nc.tensor.matmul(
                                    ps_v[sub // 2][:, sub % 2, :],
                                    negm[:, tg, sub * P:(sub + 1) * P], uv_s,
                                    start=False, stop=True)
                                nc.vector.scalar_tensor_tensor(
                                    vb[:, tt, :, 0:HD],
                                    ps_v[sub // 2][:, sub % 2, :]
                                    .rearrange("p (h e) -> p h e", h=4),
                                    rT[:, tt:tt + 1],
                                    cvb.rearrange("p (h e) -> p h e", h=4),
                                    op0=ALU.mult, op1=ALU.add)

                # ======= attention (4 local heads, banded causal+WIN) =======
                with (
                    tc.tile_pool(name="ptp", bufs=2) as ptp,
                    tc.tile_pool(name="denp", bufs=2) as denp,
                    tc.tile_pool(name="otp", bufs=2) as otp,
                    tc.tile_pool(name="pss", bufs=2, space="PSUM") as pss,
                    tc.tile_pool(name="pso", bufs=2, space="PSUM") as pso,
                ):
                    for hp in range(2):      # head pairs: local 2hp, 2hp+1
                        for qg in range(4):
                            ktg = min(KT, 4 * (qg + 1) + 1)
                            ps_o = [pso.tile([P, TOK], f32, tag=f"ps_o{hh}",
                                             name=f"ps_o{hh}") for hh in range(2)]
                            for kk in range(ktg):
                                qi_min = max(0, kk - 4 * qg - 1)
                                q0 = qi_min * P
                                ps_s = [pss.tile([P, TOK], f32, tag=f"ps_s{hh}",
                                                 name=f"ps_s{hh}")
                                        for hh in range(2)]
                                pt = [ptp.tile([P, TOK], bf16, tag=f"pt{hh}",
                                               name=f"pt{hh}") for hh in range(2)]
                                for hh in range(2):
                                    nc.tensor.matmul(
                                        ps_s[hh][:, q0:TOK],
                                        qk[hh * HD:(hh + 1) * HD, 2 + hp,
                                           kk // 4, (kk % 4) * P:(kk % 4 + 1) * P],
                                        qk[hh * HD:(hh + 1) * HD, hp, qg, q0:TOK],
                                        start=True, stop=True)
                                    for qi in range(qi_min, 4):
                                        rel = kk - (4 * qg + qi)
                                        if rel == 0 or rel == 1:
                                            m = mA if rel == 0 else mB
                                            nc.vector.tensor_add(
                                                ps_s[hh][:, qi * P:(qi + 1) * P],
                                                ps_s[hh][:, qi * P:(qi + 1) * P],
                                                m)
                                    nc.scalar.activation(
                                        out=pt[hh][:, q0:TOK],
                                        in_=ps_s[hh][:, q0:TOK], func=ACT.Exp)
                                    nc.tensor.matmul(
                                        ps_o[hh][0:HD + 1, q0:TOK],
                                        vb[:, kk, 2 * hp + hh, :],
                                        pt[hh][:, q0:TOK],
                                        start=(kk == 0), stop=(kk == ktg - 1),
                                        skip_group_check=True)
                            for hh in range(2):
                                hl = 2 * hp + hh
                                # denominator: copy to SBUF, PE-broadcast to 64
                                # partitions, reciprocal, multiply.
                                den = denp.tile([1, TOK], f32r, tag="den")
                                nc.vector.tensor_copy(den, ps_o[hh][HD:HD + 1, :])
                                dps = pss.tile([HD, TOK], f32, tag="ps_s0",
                                               name="dps")
                                nc.tensor.matmul(dps, ones64r, den,
                                                 start=True, stop=True)
                                rec = denp.tile([HD, TOK], f32, tag="rec")
                                nc.vector.reciprocal(rec, dps)
                                ot = otp.tile([HD, TOK], bf16, tag="ot")
                                nc.vector.tensor_mul(ot, ps_o[hh][0:HD, :], rec)
                                nc.sync.dma_start(
                                    cc2_in[hl * HD:(hl + 1) * HD,
                                           qg * TOK:(qg + 1) * TOK], ot)

            nc.gpsimd.collective_compute(
                "AllGather", ALU.bypass, replica_groups=groups,
                ins=[cc2_in.opt()], outs=[cc2_out.opt()])

            # =========== out-proj + residual + LN2 + FFN (my tokens) =======
            with tc.tile_pool(name="p_late", bufs=1) as p_late:
                # prefetch ALL of w1 while the AllGather runs
                w1res = p_late.tile([P, DT, F], bf16, tag="w1res")
                nc.sync.dma_start(
                    w1res, w1T[:, :].rearrange("(dt p) e -> p dt e", p=P))

                toff = (nc.partition_id() % 4) * TOK
                x1 = p_late.tile([P, DT, TOK], f32, tag="x1")
                with (
                    tc.tile_pool(name="otl", bufs=2) as otl,
                    tc.tile_pool(name="mmps2", bufs=3, space="PSUM") as mmps2,
                ):
                    OT = otl.tile([P, DT, TOK], bf16, tag="OT")
                    for dpt in range(DT):
                        nc.sync.dma_start(
                            OT[:, dpt, :],
                            cc2_out.opt()[dpt * P:(dpt + 1) * P,
                                          bass.ds(toff, TOK)])
                    for dt in range(DT):
                        xm = otl.tile([P, TOK], f32, tag="xm", name="xm")
                        nc.sync.dma_start(xm, xTm[dt * P:(dt + 1) * P, :])
                        ps = mmps2.tile([P, TOK], f32, tag="ps")
                        for dpt in range(DT):
                            nc.tensor.matmul(
                                ps, owres[:, dpt, dt * P:(dt + 1) * P],
                                OT[:, dpt, :],
                                start=(dpt == 0), stop=(dpt == DT - 1))
                        nc.vector.tensor_scalar_add(
                            x1[:, dt, :], ps, scalar1=outb_t[:, dt:dt + 1])
                        nc.vector.tensor_add(x1[:, dt, :], x1[:, dt, :], xm)

                # LN2
                h2 = p_late.tile([P, DT, TOK], bf16, tag="h2")
                with (
                    tc.tile_pool(name="lnp", bufs=1) as lnp,
                    tc.tile_pool(name="lnps", bufs=1, space="PSUM") as lnps,
                ):
                    ps_s = lnps.tile([1, TOK], f32, tag="ps_s")
                    ps_q = lnps.tile([1, TOK], f32, tag="ps_q")
                    for dt in range(DT):
                        x1b = lnp.tile([P, TOK], bf16, tag="x1b")
                        nc.vector.tensor_copy(x1b, x1[:, dt, :])
                        nc.tensor.matmul(ps_s, ones, x1b,
                                         start=(dt == 0), stop=(dt == DT - 1))
                        sq = lnp.tile([P, TOK], bf16, tag="sq")
                        nc.vector.tensor_mul(sq, x1[:, dt, :], x1[:, dt, :])
                        nc.tensor.matmul(ps_q, ones, sq,
                                         start=(dt == 0), stop=(dt == DT - 1))
                    mean = lnp.tile([1, TOK], f32, tag="mean")
                    nc.vector.tensor_scalar_mul(mean, ps_s, 1.0 / D)
                    var = lnp.tile([1, TOK], f32, tag="var")
                    nc.vector.tensor_scalar_mul(var, ps_q, 1.0 / D)
                    msq = lnp.tile([1, TOK], f32, tag="msq")
                    nc.vector.tensor_mul(msq, mean, mean)
                    nc.vector.tensor_sub(var, var, msq)
                    lnv = lnp.tile([1, TOK], f32, tag="lnv")
                    nc.scalar.activation(out=lnv, in_=var, func=ACT.Ln,
                                         bias=epst, scale=1.0)
                    rstd = lnp.tile([1, TOK], f32, tag="rstd")
                    nc.scalar.activation(out=rstd, in_=lnv, func=ACT.Exp,
                                         scale=-0.5)
                    mr = lnp.tile([1, TOK], f32, tag="mr")
                    nc.vector.tensor_mul(mr, mean, rstd)
                    st2d = dram.tile([2, TOK], f32)
                    nc.sync.dma_start(st2d[0:1, :], rstd)
                    nc.sync.dma_start(st2d[1:2, :], mr)
                    rstd_b = lnp.tile([P, TOK], f32, tag="rstd_b")
                    nc.gpsimd.dma_start(
                        rstd_b, st2d.opt()[0:1, :].partition_broadcast(P))
                    mr_b = lnp.tile([P, TOK], f32, tag="mr_b")
                    nc.gpsimd.dma_start(
                        mr_b, st2d.opt()[1:2, :].partition_broadcast(P))
                    for dt in range(DT):
                        t = lnp.tile([P, TOK], f32, tag="lnt")
                        nc.vector.tensor_mul(t, x1[:, dt, :], rstd_b)
                        nc.vector.tensor_sub(t, t, mr_b)
                        nc.vector.tensor_scalar(
                            h2[:, dt, :], t,
                            scalar1=g2t[:, dt:dt + 1], scalar2=bt2t[:, dt:dt + 1],
                            op0=ALU.mult, op1=ALU.add)

                # FFN1 from resident w1 (no weight DMAs)
                gt_ = p_late.tile([P, FT, TOK], bf16, tag="gT")
                with tc.tile_pool(name="mmps3", bufs=1, space="PSUM") as mmps3:
                    for fg in range(FT // 4):
                        ps4 = [mmps3.tile([P, TOK], f32, tag=f"psf{j}",
                                          name=f"psf{j}") for j in range(4)]
                        for dt in range(DT):
                            for j in range(4):
                                nc.tensor.matmul(
                                    ps4[j],
                                    w1res[:, dt, (fg * 4 + j) * P:
                                          (fg * 4 + j + 1) * P],
                                    h2[:, dt, :],
                                    start=(dt == 0), stop=(dt == DT - 1))
                        for j in range(4):
                            nc.scalar.activation(
                                out=gt_[:, fg * 4 + j, :], in_=ps4[j],
                                func=ACT.Gelu,
                                bias=b1_t[:, fg * 4 + j:fg * 4 + j + 1], scale=1.0)
                    # FFN2 in d-tile groups of 4, streaming w2
                    for dg in range(2):
                        ps4 = [mmps3.tile([P, TOK], f32, tag=f"psf{j}",
                                          name=f"psf{j}") for j in range(4)]
                        for ft in range(FT):
                            w4 = wpool.tile([P, 4, P], bf16, tag="w4")
                            nc.sync.dma_start(
                                w4, w2T[ft * P:(ft + 1) * P,
                                        dg * 4 * P:(dg + 1) * 4 * P]
                                .rearrange("p (e c) -> p e c", e=4))
                            for j in range(4):
                                nc.tensor.matmul(ps4[j], w4[:, j, :], gt_[:, ft, :],
                                                 start=(ft == 0),
                                                 stop=(ft == FT - 1))
                        for j in range(4):
                            dt = dg * 4 + j
                            o = epil.tile([P, TOK], f32, tag="ep")
                            nc.vector.tensor_scalar_add(
                                o, ps4[j], scalar1=b2_t[:, dt:dt + 1])
                            nc.vector.tensor_add(o, o, x1[:, dt, :])
                            nc.sync.dma_start(yT[dt * P:(dt + 1) * P, :], o)

    return nc


@functools.lru_cache(maxsize=1)
def _build():
    import concourse.tile as tile
    import concourse.mybir as mybir
    from concourse import bacc

    nc = bacc.Bacc("TRN2")
    _emit(nc, tile, mybir)
    nc.compile()
    return nc


def _host_prep(inputs):
    """Transpose weights/x; fold LN1 gamma/beta + 1/sqrt(hd) into QKV."""
    import ml_dtypes

    bf = ml_dtypes.bfloat16
    x = np.asarray(inputs["x"], dtype=np.float32)
    ipw = np.asarray(inputs["in_proj_w"], dtype=np.float32)
    ipb = np.asarray(inputs["in_proj_b"], dtype=np.float32)
    g1 = np.asarray(inputs["ln1_g"], dtype=np.float32)
    bt1 = np.asarray(inputs["ln1_b"], dtype=np.float32)
    sc = 1.0 / np.sqrt(np.float32(HD))
    wsc = np.concatenate([ipw[:D] * sc, ipw[D:]], axis=0)   # q-scale folded
    bsc = np.concatenate([ipb[:D] * sc, ipb[D:]], axis=0)
    wfold = wsc * g1[None, :]                               # LN1 gamma folded
    ufold = wfold.sum(axis=1)                               # [3D]
    cfold = wsc @ bt1 + bsc                                 # LN1 beta folded

    shared = {
        "outwT": np.ascontiguousarray(
            np.asarray(inputs["out_w"], np.float32).T.astype(bf)),
        "w1T": np.ascontiguousarray(
            np.asarray(inputs["w1"], np.float32).T.astype(bf)),
        "w2T": np.ascontiguousarray(
            np.asarray(inputs["w2"], np.float32).T.astype(bf)),
        "outb": np.ascontiguousarray(np.asarray(inputs["out_b"], np.float32)),
        "b1": np.ascontiguousarray(np.asarray(inputs["b1"], np.float32)),
        "b2": np.ascontiguousarray(np.asarray(inputs["b2"], np.float32)),
        "ln2g": np.ascontiguousarray(np.asarray(inputs["ln2_g"], np.float32)),
        "ln2b": np.ascontiguousarray(np.asarray(inputs["ln2_b"], np.float32)),
    }
    ii, jj = np.arange(P)[:, None], np.arange(P)[None, :]
    shared["maskA"] = np.where(ii <= jj + WIN, 0.0, NEG).astype(np.float32)
    shared["maskB"] = np.where(ii <= jj - (P - WIN), 0.0, NEG).astype(np.float32)

    xT = [np.ascontiguousarray(x[b].T) for b in range(B)]
    xTb = [np.ascontiguousarray(t.astype(bf)) for t in xT]

    in_maps = []
    for c in range(NCORES):
        b, r = c // 4, c % 4
        qs = slice(EM * r, EM * (r + 1))            # my q rows in [0, D)
        ks = slice(D + EM * r, D + EM * (r + 1))
        vs = slice(2 * D + EM * r, 2 * D + EM * (r + 1))
        m = dict(shared)
        m["xTf"] = xTb[b]
        m["xTm"] = np.ascontiguousarray(xT[b][:, TOK * r: TOK * (r + 1)])
        m["wqkvT"] = np.ascontiguousarray(
            np.concatenate([wfold[qs], wfold[ks], wfold[vs]], axis=0).T.astype(bf))
        m["uqk"] = np.ascontiguousarray(
            np.concatenate([ufold[qs], ufold[ks]]).reshape(1, -1).astype(bf))
        m["uv"] = np.ascontiguousarray(ufold[vs].reshape(1, -1).astype(bf))
        m["cqk"] = np.ascontiguousarray(
            np.concatenate([cfold[qs], cfold[ks]]))
        m["cv"] = np.ascontiguousarray(cfold[vs].reshape(1, -1))
        in_maps.append(m)
    return in_maps


def kernel(**inputs) -> np.ndarray:
    from concourse.bass_utils import run_bass_kernel_spmd

    nc = _build()
    in_maps = _host_prep(inputs)
    res = run_bass_kernel_spmd(nc, in_maps, core_ids=list(range(NCORES)),
                               trace=False)
    y = np.empty((B, T, D), dtype=np.float32)
    for c in range(NCORES):
        b, r = c // 4, c % 4
        y[b, TOK * r: TOK * (r + 1), :] = res.results[c]["yT"].T
    return y


# revision 17
# speedup vs baseline: 1.0539x; 1.0539x over previous
"""Trainium2 Bass kernel for a pre-norm look-ahead-window transformer block.

Problem: y = x + FFN(LN2(x + Attn(LN1(x)))) with banded-causal attention
(key j visible to query i iff j <= i + WIN), B=2, T=2048, D=1024, H=16.

Distribution over 8 NeuronCores (one SPMD program, uniform across cores):
  core c: batch b = c//4, rank r = c%4 in the 4-core batch group.
  - residual / out-proj / FFN are token-sharded (512 tokens per core).
  - QKV + attention are head-sharded (4 heads, all 2048 tokens), which keeps
    the banded-attention loop structure identical on every core. LN1 is
    folded into the QKV weights on the host (W' = W*g per column, rank-1
    mean correction via a K=1 matmul, per-token rstd scale in the epilogue);
    LN1 stats for all tokens are computed locally on every core (cheaper
    than a collective's latency floor).
  - ONE 4-rank AllGather moves the attention output O^T (1 MB bf16); all
    FFN1 weights are prefetched into SBUF underneath it. The post-gather
    token slice is selected with a partition-id-driven dynamic DMA offset.

Everything runs in a transposed [feature, token] layout so matmuls contract
over SBUF partitions; the host pre-transposes x / weights and transposes
the per-core [D, 512] output shards back. Matmul inputs are bf16 (full PE
rate); residuals and PSUM accumulation stay fp32.
"""

import functools

import numpy as np

B, T, D, H, HD, WIN, F = 2, 2048, 1024, 16, 64, 10, 4096
P = 128
NCORES = 8
TOK = 512          # tokens per core for the token-sharded phases
DT = D // P        # 8 d-tiles
FT = F // P        # 32 f-tiles
KT = T // P        # 16 key tiles
EM = 2 * P         # 256 e-rows per core per projection (4 heads x 64)
EPS = 1e-5
NEG = -30000.0     # additive mask; exp(NEG) == 0 exactly on ACT


def _emit(nc, tile, mybir):
    import concourse.bass as bass

    f32 = mybir.dt.float32
    f32r = mybir.dt.float32r
    bf16 = mybir.dt.bfloat16
    ALU = mybir.AluOpType
    ACT = mybir.ActivationFunctionType

    # ---- kernel I/O ----
    xTf = nc.dram_tensor("xTf", [D, T], bf16, kind="ExternalInput")
    xTm = nc.dram_tensor("xTm", [D, TOK], f32, kind="ExternalInput")
    wqkvT = nc.dram_tensor("wqkvT", [D, 3 * EM], bf16, kind="ExternalInput")
    uqk = nc.dram_tensor("uqk", [1, 2 * EM], bf16, kind="ExternalInput")
    uv = nc.dram_tensor("uv", [1, EM], bf16, kind="ExternalInput")
    cqk = nc.dram_tensor("cqk", [2 * EM], f32, kind="ExternalInput")
    cv = nc.dram_tensor("cv", [1, EM], f32, kind="ExternalInput")
    outwT = nc.dram_tensor("outwT", [D, D], bf16, kind="ExternalInput")
    w1T = nc.dram_tensor("w1T", [D, F], bf16, kind="ExternalInput")
    w2T = nc.dram_tensor("w2T", [F, D], bf16, kind="ExternalInput")
    outb = nc.dram_tensor("outb", [D], f32, kind="ExternalInput")
    b1 = nc.dram_tensor("b1", [F], f32, kind="ExternalInput")
    b2 = nc.dram_tensor("b2", [D], f32, kind="ExternalInput")
    ln2g = nc.dram_tensor("ln2g", [D], f32, kind="ExternalInput")
    ln2b = nc.dram_tensor("ln2b", [D], f32, kind="ExternalInput")
    maskA = nc.dram_tensor("maskA", [P, P], f32, kind="ExternalInput")
    maskB = nc.dram_tensor("maskB", [P, P], f32, kind="ExternalInput")
    yT = nc.dram_tensor("yT", [D, TOK], f32, kind="ExternalOutput")

    groups = [[0, 1, 2, 3], [4, 5, 6, 7]]

    with tile.TileContext(nc) as tc:
        from contextlib import ExitStack

        with ExitStack() as ctx:
            consts = ctx.enter_context(tc.tile_pool(name="consts", bufs=1))
            epil = ctx.enter_context(tc.tile_pool(name="epil", bufs=4))
            wpool = ctx.enter_context(tc.tile_pool(name="wpool", bufs=6))
            dram = ctx.enter_context(tc.tile_pool(name="dram", bufs=1, space="DRAM"))

            # ---- constants ----
            ones_f = consts.tile([P, 1], f32)
            nc.vector.memset(ones_f, 1.0)
            ones = consts.tile([P, 1], bf16)
            nc.vector.tensor_copy(ones, ones_f)
            ones64r = consts.tile([1, HD], f32r)
            nc.vector.tensor_copy(ones64r, ones_f[0:1, 0:1].to_broadcast((1, HD)))
            mA = consts.tile([P, P], f32)
            nc.sync.dma_start(mA, maskA[:, :])
            mB = consts.tile([P, P], f32)
            nc.sync.dma_start(mB, maskB[:, :])

            def load_pp(dramvec, n):  # [n*P] -> [P, n] per-partition layout
                t = consts.tile([P, n], f32, tag=f"pp_{dramvec.name}")
                nc.sync.dma_start(t, dramvec[:].rearrange("(n p) -> p n", p=P))
                return t

            cqk_t = load_pp(cqk, 4)
            outb_t = load_pp(outb, DT)
            b1_t = load_pp(b1, FT)
            b2_t = load_pp(b2, DT)
            g2t = load_pp(ln2g, DT)
            bt2t = load_pp(ln2b, DT)
            epst = consts.tile([1, 1], f32)
            nc.vector.memset(epst, EPS)
            uqk_s = consts.tile([1, 2 * EM], bf16)
            nc.sync.dma_start(uqk_s, uqk[:, :])
            uv_s = consts.tile([1, EM], bf16)
            nc.sync.dma_start(uv_s, uv[:, :])
            cvb = consts.tile([P, EM], f32)
            nc.gpsimd.dma_start(cvb, cv[:, :].partition_broadcast(P))
            wvres = consts.tile([P, DT, EM], bf16)   # V weights, resident
            nc.sync.dma_start(
                wvres, wqkvT[:, 2 * EM:].rearrange("(dt p) e -> p dt e", p=P))
            wqkres = consts.tile([P, DT, 4, P], bf16)  # Q/K weights, resident
            nc.sync.dma_start(
                wqkres, wqkvT[:, 0:2 * EM]
                .rearrange("(dt p) (e c) -> p dt e c", p=P, e=4))
            owres = consts.tile([P, DT, D], bf16)    # out-proj weights, resident
            nc.sync.dma_start(
                owres, outwT[:, :].rearrange("(dp p) e -> p dp e", p=P))

            cc2_in = dram.tile([EM, T], bf16)
            cc2_out = dram.tile([4 * EM, T], bf16)
            stats_d = dram.tile([2, 4, TOK], f32)    # [0]=rstd, [1]=negm rows

            # q/k/v for my 4 heads over all tokens, then banded attention
            with tc.tile_pool(name="p_qkv", bufs=1) as p_qkv:
                xfull = p_qkv.tile([P, DT, 4, TOK], bf16, tag="xfull")
                nc.sync.dma_start(
                    xfull,
                    xTf[:, :].rearrange("(dt p) (tg t) -> p dt tg t", p=P, t=TOK))
                # qk[:, 0:2, tg, :] = Q^T e-tiles, qk[:, 2:4, tg, :] = K^T
                qk = p_qkv.tile([P, 4, 4, TOK], bf16, tag="qk")
                vb = p_qkv.tile([P, KT, 4, HD + 1], bf16, tag="vb")
                ones_kh = p_qkv.tile([P, KT * 4], f32, tag="ones_kh")
                nc.vector.memset(ones_kh, 1.0)
                nc.vector.tensor_copy(
                    vb[:, :, :, HD],
                    ones_kh.rearrange("p (k h) -> p k h", h=4))

                with tc.tile_pool(name="qkvc", bufs=1) as qkvc:
                    # ---- LN1 stats for ALL tokens, locally ----
                    negm = qkvc.tile([1, 4, TOK], bf16)
                    with (
                        tc.tile_pool(name="st", bufs=2) as st,
                        tc.tile_pool(name="stps", bufs=2, space="PSUM") as stps,
                    ):
                        for tg in range(4):
                            ps_s = stps.tile([1, TOK], f32, tag="ps_s",
                                             name="ps_s")
                            ps_q = stps.tile([1, TOK], f32, tag="ps_q",
                                             name="ps_q")
                            for dt in range(DT):
                                nc.tensor.matmul(ps_s, ones, xfull[:, dt, tg, :],
                                                 start=(dt == 0),
                                                 stop=(dt == DT - 1))
                                sq = st.tile([P, TOK], bf16, tag="sq")
                                nc.vector.tensor_mul(sq, xfull[:, dt, tg, :],
                                                     xfull[:, dt, tg, :])
                                nc.tensor.matmul(ps_q, ones, sq,
                                                 start=(dt == 0),
                                                 stop=(dt == DT - 1))
                            mean = st.tile([1, TOK], f32, tag="mean")
                            nc.vector.tensor_scalar_mul(mean, ps_s, 1.0 / D)
                            var = st.tile([1, TOK], f32, tag="var")
                            nc.vector.tensor_scalar_mul(var, ps_q, 1.0 / D)
                            msq = st.tile([1, TOK], f32, tag="msq")
                            nc.vector.tensor_mul(msq, mean, mean)
                            nc.vector.tensor_sub(var, var, msq)
                            lnv = st.tile([1, TOK], f32, tag="lnv")
                            nc.scalar.activation(out=lnv, in_=var, func=ACT.Ln,
                                                 bias=epst, scale=1.0)
                            rstd1 = st.tile([1, TOK], f32, tag="rstd1")
                            nc.scalar.activation(out=rstd1, in_=lnv,
                                                 func=ACT.Exp, scale=-0.5)
                            nc.sync.dma_start(stats_d[0, tg, :], rstd1)
                            nc.vector.tensor_scalar_mul(negm[:, tg, :], mean,
                                                        -1.0)
                    rT = qkvc.tile([P, KT], f32)          # rstd, token-partition
                    nc.sync.dma_start(
                        rT, stats_d.opt()[0].rearrange("(tt p) -> p tt", p=P))
                    rbc = qkvc.tile([P, 4, TOK], f32)     # rstd bcast over parts
                    nc.gpsimd.dma_start(
                        rbc, stats_d.opt()[0:1].partition_broadcast(P))

                    # ---- QKV matmuls (all weights + x resident in SBUF) ----
                    with (
                        tc.tile_pool(name="qkps", bufs=1, space="PSUM") as qkps,
                        tc.tile_pool(name="vps", bufs=1, space="PSUM") as vps,
                    ):
                        for tg in range(4):
                            ps_qk = [qkps.tile([P, TOK], f32, tag=f"ps{e}",
                                               name=f"ps{e}") for e in range(4)]
                            ps_v = [vps.tile([P, EM], f32, tag=f"psv{s}",
                                             name=f"psv{s}") for s in range(4)]
                            for dt in range(DT):
                                for et in range(4):
                                    nc.tensor.matmul(
                                        ps_qk[et], wqkres[:, dt, et, :],
                                        xfull[:, dt, tg, :],
                                        start=(dt == 0), stop=False)
                                for sub in range(4):
                                    nc.tensor.matmul(
                                        ps_v[sub],
                                        xfull[:, dt, tg, sub * P:(sub + 1) * P],
                                        wvres[:, dt, :],
                                        start=(dt == 0), stop=False)
                            for et in range(4):
                                nc.tensor.matmul(
                                    ps_qk[et], uqk_s[:, et * P:(et + 1) * P],
                                    negm[:, tg, :], start=False, stop=True)
                                t = epil.tile([P, TOK], f32, tag="ep")
                                nc.vector.tensor_mul(t, ps_qk[et], rbc[:, tg, :])
                                nc.vector.tensor_scalar_add(
                                    qk[:, et, tg, :], t,
                                    scalar1=cqk_t[:, et:et + 1])
                            for sub in range(4):
                                tt = tg * 4 + sub
                                nc.tensor.matmul(
                                    ps_v[sub],
                                    negm[:, tg, sub * P:(sub + 1) * P], uv_s,
                                    start=False, stop=True)
                                nc.vector.scalar_tensor_tensor(
                                    vb[:, tt, :, 0:HD],
                                    ps_v[sub]
                                    .rearrange("p (h e) -> p h e", h=4),
                                    rT[:, tt:tt + 1],
                                    cvb.rearrange("p (h e) -> p h e", h=4),
                                    op0=ALU.mult, op1=ALU.add)

                # ======= attention (4 local heads, banded causal+WIN) =======
                with (
                    tc.tile_pool(name="ptp", bufs=2) as ptp,
                    tc.tile_pool(name="denp", bufs=2) as denp,
                    tc.tile_pool(name="otp", bufs=2) as otp,
                    tc.tile_pool(name="pss", bufs=2, space="PSUM") as pss,
                    tc.tile_pool(name="pso", bufs=2, space="PSUM") as pso,
                ):
                    for hp in range(2):      # head pairs: local 2hp, 2hp+1
                        for qg in range(4):
                            ktg = min(KT, 4 * (qg + 1) + 1)
                            ps_o = [pso.tile([P, TOK], f32, tag=f"ps_o{hh}",
                                             name=f"ps_o{hh}") for hh in range(2)]
                            for kk in range(ktg):
                                qi_min = max(0, kk - 4 * qg - 1)
                                q0 = qi_min * P
                                ps_s = [pss.tile([P, TOK], f32, tag=f"ps_s{hh}",
                                                 name=f"ps_s{hh}")
                                        for hh in range(2)]
                                pt = [ptp.tile([P, TOK], bf16, tag=f"pt{hh}",
                                               name=f"pt{hh}") for hh in range(2)]
                                for hh in range(2):
                                    nc.tensor.matmul(
                                        ps_s[hh][:, q0:TOK],
                                        qk[hh * HD:(hh + 1) * HD, 2 + hp,
                                           kk // 4, (kk % 4) * P:(kk % 4 + 1) * P],
                                        qk[hh * HD:(hh + 1) * HD, hp, qg, q0:TOK],
                                        start=True, stop=True)
                                    for qi in range(qi_min, 4):
                                        rel = kk - (4 * qg + qi)
                                        if rel == 0 or rel == 1:
                                            m = mA if rel == 0 else mB
                                            nc.vector.tensor_add(
                                                ps_s[hh][:, qi * P:(qi + 1) * P],
                                                ps_s[hh][:, qi * P:(qi + 1) * P],
                                                m)
                                    nc.scalar.activation(
                                        out=pt[hh][:, q0:TOK],
                                        in_=ps_s[hh][:, q0:TOK], func=ACT.Exp)
                                    nc.tensor.matmul(
                                        ps_o[hh][0:HD + 1, q0:TOK],
                                        vb[:, kk, 2 * hp + hh, :],
                                        pt[hh][:, q0:TOK],
                                        start=(kk == 0), stop=(kk == ktg - 1),
                                        skip_group_check=True)
                            for hh in range(2):
                                hl = 2 * hp + hh
                                # denominator: copy to SBUF, PE-broadcast to 64
                                # partitions, reciprocal, multiply.
                                den = denp.tile([1, TOK], f32r, tag="den")
                                nc.vector.tensor_copy(den, ps_o[hh][HD:HD + 1, :])
                                dps = pss.tile([HD, TOK], f32, tag="ps_s0",
                                               name="dps")
                                nc.tensor.matmul(dps, ones64r, den,
                                                 start=True, stop=True)
                                rec = denp.tile([HD, TOK], f32, tag="rec")
                                nc.vector.reciprocal(rec, dps)
                                ot = otp.tile([HD, TOK], bf16, tag="ot")
                                nc.vector.tensor_mul(ot, ps_o[hh][0:HD, :], rec)
                                nc.sync.dma_start(
                                    cc2_in[hl * HD:(hl + 1) * HD,
                                           qg * TOK:(qg + 1) * TOK], ot)

            nc.gpsimd.collective_compute(
                "AllGather", ALU.bypass, replica_groups=groups,
                ins=[cc2_in.opt()], outs=[cc2_out.opt()])

            # =========== out-proj + residual + LN2 + FFN (my tokens) =======
            with tc.tile_pool(name="p_late", bufs=1) as p_late:
                # prefetch ALL of w1 while the AllGather runs
                w1res = p_late.tile([P, DT, F], bf16, tag="w1res")
                nc.sync.dma_start(
                    w1res, w1T[:, :].rearrange("(dt p) e -> p dt e", p=P))

                toff = (nc.partition_id() % 4) * TOK
                x1 = p_late.tile([P, DT, TOK], f32, tag="x1")
                with (
                    tc.tile_pool(name="otl", bufs=2) as otl,
                    tc.tile_pool(name="mmps2", bufs=3, space="PSUM") as mmps2,
                ):
                    OT = otl.tile([P, DT, TOK], bf16, tag="OT")
                    for dpt in range(DT):
                        nc.sync.dma_start(
                            OT[:, dpt, :],
                            cc2_out.opt()[dpt * P:(dpt + 1) * P,
                                          bass.ds(toff, TOK)])
                    for dt in range(DT):
                        xm = otl.tile([P, TOK], f32, tag="xm", name="xm")
                        nc.sync.dma_start(xm, xTm[dt * P:(dt + 1) * P, :])
                        ps = mmps2.tile([P, TOK], f32, tag="ps")
                        for dpt in range(DT):
                            nc.tensor.matmul(
                                ps, owres[:, dpt, dt * P:(dt + 1) * P],
                                OT[:, dpt, :],
                                start=(dpt == 0), stop=(dpt == DT - 1))
                        nc.vector.tensor_scalar_add(
                            x1[:, dt, :], ps, scalar1=outb_t[:, dt:dt + 1])
                        nc.vector.tensor_add(x1[:, dt, :], x1[:, dt, :], xm)

                # LN2
                h2 = p_late.tile([P, DT, TOK], bf16, tag="h2")
                with (
                    tc.tile_pool(name="lnp", bufs=1) as lnp,
                    tc.tile_pool(name="lnps", bufs=1, space="PSUM") as lnps,
                ):
                    ps_s = lnps.tile([1, TOK], f32, tag="ps_s")
                    ps_q = lnps.tile([1, TOK], f32, tag="ps_q")
                    for dt in range(DT):
                        x1b = lnp.tile([P, TOK], bf16, tag="x1b")
                        nc.vector.tensor_copy(x1b, x1[:, dt, :])
                        nc.tensor.matmul(ps_s, ones, x1b,
                                         start=(dt == 0), stop=(dt == DT - 1))
                        sq = lnp.tile([P, TOK], bf16, tag="sq")
                        nc.vector.tensor_mul(sq, x1[:, dt, :], x1[:, dt, :])
                        nc.tensor.matmul(ps_q, ones, sq,
                                         start=(dt == 0), stop=(dt == DT - 1))
                    mean = lnp.tile([1, TOK], f32, tag="mean")
                    nc.vector.tensor_scalar_mul(mean, ps_s, 1.0 / D)
                    var = lnp.tile([1, TOK], f32, tag="var")
                    nc.vector.tensor_scalar_mul(var, ps_q, 1.0 / D)
                    msq = lnp.tile([1, TOK], f32, tag="msq")
                    nc.vector.tensor_mul(msq, mean, mean)
                    nc.vector.tensor_sub(var, var, msq)
                    lnv = lnp.tile([1, TOK], f32, tag="lnv")
                    nc.scalar.activation(out=lnv, in_=var, func=ACT.Ln,
                                         bias=epst, scale=1.0)
                    rstd = lnp.tile([1, TOK], f32, tag="rstd")
                    nc.scalar.activation(out=rstd, in_=lnv, func=ACT.Exp,
                                         scale=-0.5)
                    mr = lnp.tile([1, TOK], f32, tag="mr")
                    nc.vector.tensor_mul(mr, mean, rstd)
                    st2d = dram.tile([2, TOK], f32)
                    nc.sync.dma_start(st2d[0:1, :], rstd)
                    nc.sync.dma_start(st2d[1:2, :], mr)
                    rstd_b = lnp.tile([P, TOK], f32, tag="rstd_b")
                    nc.gpsimd.dma_start(
                        rstd_b, st2d.opt()[0:1, :].partition_broadcast(P))
                    mr_b = lnp.tile([P, TOK], f32, tag="mr_b")
                    nc.gpsimd.dma_start(
                        mr_b, st2d.opt()[1:2, :].partition_broadcast(P))
                    for dt in range(DT):
                        t = lnp.tile([P, TOK], f32, tag="lnt")
                        nc.vector.tensor_mul(t, x1[:, dt, :], rstd_b)
                        nc.vector.tensor_sub(t, t, mr_b)
                        nc.vector.tensor_scalar(
                            h2[:, dt, :], t,
                            scalar1=g2t[:, dt:dt + 1], scalar2=bt2t[:, dt:dt + 1],
                            op0=ALU.mult, op1=ALU.add)

                # FFN1 from resident w1 (no weight DMAs)
                gt_ = p_late.tile([P, FT, TOK], bf16, tag="gT")
                with tc.tile_pool(name="mmps3", bufs=1, space="PSUM") as mmps3:
                    for fg in range(FT // 4):
                        ps4 = [mmps3.tile([P, TOK], f32, tag=f"psf{j}",
                                          name=f"psf{j}") for j in range(4)]
                        for dt in range(DT):
                            for j in range(4):
                                nc.tensor.matmul(
                                    ps4[j],
                                    w1res[:, dt, (fg * 4 + j) * P:
                                          (fg * 4 + j + 1) * P],
                                    h2[:, dt, :],
                                    start=(dt == 0), stop=(dt == DT - 1))
                        for j in range(4):
                            nc.scalar.activation(
                                out=gt_[:, fg * 4 + j, :], in_=ps4[j],
                                func=ACT.Gelu,
                                bias=b1_t[:, fg * 4 + j:fg * 4 + j + 1], scale=1.0)
                    # FFN2 in d-tile groups of 4, streaming w2
                    for dg in range(2):
                        ps4 = [mmps3.tile([P, TOK], f32, tag=f"psf{j}",
                                          name=f"psf{j}") for j in range(4)]
                        for ft in range(FT):
                            w4 = wpool.tile([P, 4, P], bf16, tag="w4")
                            nc.sync.dma_start(
                                w4, w2T[ft * P:(ft + 1) * P,
                                        dg * 4 * P:(dg + 1) * 4 * P]
                                .rearrange("p (e c) -> p e c", e=4))
                            for j in range(4):
                                nc.tensor.matmul(ps4[j], w4[:, j, :], gt_[:, ft, :],
                                                 start=(ft == 0),
                                                 stop=(ft == FT - 1))
                        for j in range(4):
                            dt = dg * 4 + j
                            o = epil.tile([P, TOK], f32, tag="ep")
                            nc.vector.tensor_scalar_add(
                                o, ps4[j], scalar1=b2_t[:, dt:dt + 1])
                            nc.vector.tensor_add(o, o, x1[:, dt, :])
                            nc.sync.dma_start(yT[dt * P:(dt + 1) * P, :], o)

    return nc


@functools.lru_cache(maxsize=1)
def _build():
    import concourse.tile as tile
    import concourse.mybir as mybir
    from concourse import bacc

    nc = bacc.Bacc("TRN2")
    _emit(nc, tile, mybir)
    nc.compile()
    return nc


def _host_prep(inputs):
    """Transpose weights/x; fold LN1 gamma/beta + 1/sqrt(hd) into QKV."""
    import ml_dtypes

    bf = ml_dtypes.bfloat16
    x = np.asarray(inputs["x"], dtype=np.float32)
    ipw = np.asarray(inputs["in_proj_w"], dtype=np.float32)
    ipb = np.asarray(inputs["in_proj_b"], dtype=np.float32)
    g1 = np.asarray(inputs["ln1_g"], dtype=np.float32)
    bt1 = np.asarray(inputs["ln1_b"], dtype=np.float32)
    sc = 1.0 / np.sqrt(np.float32(HD))
    wsc = np.concatenate([ipw[:D] * sc, ipw[D:]], axis=0)   # q-scale folded
    bsc = np.concatenate([ipb[:D] * sc, ipb[D:]], axis=0)
    wfold = wsc * g1[None, :]                               # LN1 gamma folded
    ufold = wfold.sum(axis=1)                               # [3D]
    cfold = wsc @ bt1 + bsc                                 # LN1 beta folded

    shared = {
        "outwT": np.ascontiguousarray(
            np.asarray(inputs["out_w"], np.float32).T.astype(bf)),
        "w1T": np.ascontiguousarray(
            np.asarray(inputs["w1"], np.float32).T.astype(bf)),
        "w2T": np.ascontiguousarray(
            np.asarray(inputs["w2"], np.float32).T.astype(bf)),
        "outb": np.ascontiguousarray(np.asarray(inputs["out_b"], np.float32)),
        "b1": np.ascontiguousarray(np.asarray(inputs["b1"], np.float32)),
        "b2": np.ascontiguousarray(np.asarray(inputs["b2"], np.float32)),
        "ln2g": np.ascontiguousarray(np.asarray(inputs["ln2_g"], np.float32)),
        "ln2b": np.ascontiguousarray(np.asarray(inputs["ln2_b"], np.float32)),
    }
    ii, jj = np.arange(P)[:, None], np.arange(P)[None, :]
    shared["maskA"] = np.where(ii <= jj + WIN, 0.0, NEG).astype(np.float32)
    shared["maskB"] = np.where(ii <= jj - (P - WIN), 0.0, NEG).astype(np.float32)

    xT = [np.ascontiguousarray(x[b].T) for b in range(B)]
    xTb = [np.ascontiguousarray(t.astype(bf)) for t in xT]

    in_maps = []
    for c in range(NCORES):
        b, r = c // 4, c % 4
        qs = slice(EM * r, EM * (r + 1))            # my q rows in [0, D)
        ks = slice(D + EM * r, D + EM * (r + 1))
        vs = slice(2 * D + EM * r, 2 * D + EM * (r + 1))
        m = dict(shared)
        m["xTf"] = xTb[b]
        m["xTm"] = np.ascontiguousarray(xT[b][:, TOK * r: TOK * (r + 1)])
        m["wqkvT"] = np.ascontiguousarray(
            np.concatenate([wfold[qs], wfold[ks], wfold[vs]], axis=0).T.astype(bf))
        m["uqk"] = np.ascontiguousarray(
            np.concatenate([ufold[qs], ufold[ks]]).reshape(1, -1).astype(bf))
        m["uv"] = np.ascontiguousarray(ufold[vs].reshape(1, -1).astype(bf))
        m["cqk"] = np.ascontiguousarray(
            np.concatenate([cfold[qs], cfold[ks]]))
        m["cv"] = np.ascontiguousarray(cfold[vs].reshape(1, -1))
        in_maps.append(m)
    return in_maps


def kernel(**inputs) -> np.ndarray:
    from concourse.bass_utils import run_bass_kernel_spmd

    nc = _build()
    in_maps = _host_prep(inputs)
    res = run_bass_kernel_spmd(nc, in_maps, core_ids=list(range(NCORES)),
                               trace=False)
    y = np.empty((B, T, D), dtype=np.float32)
    for c in range(NCORES):
        b, r = c // 4, c % 4
        y[b, TOK * r: TOK * (r + 1), :] = res.results[c]["yT"].T
    return y


# revision 24
# speedup vs baseline: 1.1282x; 1.0706x over previous
"""Trainium2 Bass kernel for a pre-norm look-ahead-window transformer block.

Problem: y = x + FFN(LN2(x + Attn(LN1(x)))) with banded-causal attention
(key j visible to query i iff j <= i + WIN), B=2, T=2048, D=1024, H=16.

Distribution over 8 NeuronCores (one SPMD program, uniform across cores):
  core c: batch b = c//4, rank r = c%4 in the 4-core batch group.
  - residual / out-proj / FFN are token-sharded (512 tokens per core).
  - QKV + attention are head-sharded (4 heads, all 2048 tokens), which keeps
    the banded-attention loop structure identical on every core. LN1 is
    folded into the QKV weights on the host (W' = W*g per column, rank-1
    mean correction via a K=1 matmul, per-token rstd scale in the epilogue);
    LN1 stats for all tokens are computed locally on every core (cheaper
    than a collective's latency floor).
  - ONE 4-rank AllGather moves the attention output O^T (1 MB bf16); all
    FFN1 weights are prefetched into SBUF underneath it. The post-gather
    token slice is selected with a partition-id-driven dynamic DMA offset.

Everything runs in a transposed [feature, token] layout so matmuls contract
over SBUF partitions; the host pre-transposes x / weights and transposes
the per-core [D, 512] output shards back. Matmul inputs are bf16 (full PE
rate); residuals and PSUM accumulation stay fp32.
"""

import functools

import numpy as np

B, T, D, H, HD, WIN, F = 2, 2048, 1024, 16, 64, 10, 4096
P = 128
NCORES = 8
TOK = 512          # tokens per core for the token-sharded phases
DT = D // P        # 8 d-tiles
FT = F // P        # 32 f-tiles
KT = T // P        # 16 key tiles
EM = 2 * P         # 256 e-rows per core per projection (4 heads x 64)
EPS = 1e-5
NEG = -30000.0     # additive mask; exp(NEG) == 0 exactly on ACT


def _emit(nc, tile, mybir):
    import concourse.bass as bass

    f32 = mybir.dt.float32
    f32r = mybir.dt.float32r
    bf16 = mybir.dt.bfloat16
    ALU = mybir.AluOpType
    ACT = mybir.ActivationFunctionType

    # ---- kernel I/O ----
    xTf = nc.dram_tensor("xTf", [D, T], bf16, kind="ExternalInput")
    xTm = nc.dram_tensor("xTm", [D, TOK], f32, kind="ExternalInput")
    wqkvT = nc.dram_tensor("wqkvT", [D, 3 * EM], bf16, kind="ExternalInput")
    uqk = nc.dram_tensor("uqk", [1, 2 * EM], bf16, kind="ExternalInput")
    uv = nc.dram_tensor("uv", [1, EM], bf16, kind="ExternalInput")
    cqk = nc.dram_tensor("cqk", [2 * EM], f32, kind="ExternalInput")
    cv = nc.dram_tensor("cv", [1, EM], f32, kind="ExternalInput")
    outwT = nc.dram_tensor("outwT", [D, D], bf16, kind="ExternalInput")
    w1T = nc.dram_tensor("w1T", [D, F], bf16, kind="ExternalInput")
    w2T = nc.dram_tensor("w2T", [F, D], bf16, kind="ExternalInput")
    outb = nc.dram_tensor("outb", [D], f32, kind="ExternalInput")
    b1 = nc.dram_tensor("b1", [F], f32, kind="ExternalInput")
    b2 = nc.dram_tensor("b2", [D], f32, kind="ExternalInput")
    ln2g = nc.dram_tensor("ln2g", [D], f32, kind="ExternalInput")
    ln2b = nc.dram_tensor("ln2b", [D], f32, kind="ExternalInput")
    maskA = nc.dram_tensor("maskA", [P, P], f32, kind="ExternalInput")
    maskB = nc.dram_tensor("maskB", [P, P], f32, kind="ExternalInput")
    yT = nc.dram_tensor("yT", [D, TOK], f32, kind="ExternalOutput")

    groups = [[0, 1, 2, 3], [4, 5, 6, 7]]

    with tile.TileContext(nc) as tc:
        from contextlib import ExitStack

        with ExitStack() as ctx:
            consts = ctx.enter_context(tc.tile_pool(name="consts", bufs=1))
            epil = ctx.enter_context(tc.tile_pool(name="epil", bufs=4))
            wpool = ctx.enter_context(tc.tile_pool(name="wpool", bufs=2))
            dram = ctx.enter_context(tc.tile_pool(name="dram", bufs=1, space="DRAM"))

            # ---- constants ----
            ones_f = consts.tile([P, 1], f32)
            nc.vector.memset(ones_f, 1.0)
            ones = consts.tile([P, 1], bf16)
            nc.vector.tensor_copy(ones, ones_f)
            ones64r = consts.tile([1, HD], f32r)
            nc.vector.tensor_copy(ones64r, ones_f[0:1, 0:1].to_broadcast((1, HD)))
            mA = consts.tile([P, P], f32)
            nc.sync.dma_start(mA, maskA[:, :])
            mB = consts.tile([P, P], f32)
            nc.sync.dma_start(mB, maskB[:, :])

            def load_pp(dramvec, n):  # [n*P] -> [P, n] per-partition layout
                t = consts.tile([P, n], f32, tag=f"pp_{dramvec.name}")
                nc.sync.dma_start(t, dramvec[:].rearrange("(n p) -> p n", p=P))
                return t

            cqk_t = load_pp(cqk, 4)
            outb_t = load_pp(outb, DT)
            b1_t = load_pp(b1, FT)
            b2_t = load_pp(b2, DT)
            g2t = load_pp(ln2g, DT)
            bt2t = load_pp(ln2b, DT)
            epst = consts.tile([1, 1], f32)
            nc.vector.memset(epst, EPS)
            uqk_s = consts.tile([1, 2 * EM], bf16)
            nc.sync.dma_start(uqk_s, uqk[:, :])
            uv_s = consts.tile([1, EM], bf16)
            nc.sync.dma_start(uv_s, uv[:, :])
            cvb = consts.tile([P, EM], f32)
            nc.gpsimd.dma_start(cvb, cv[:, :].partition_broadcast(P))
            wvres = consts.tile([P, DT, EM], bf16)   # V weights, resident
            nc.sync.dma_start(
                wvres, wqkvT[:, 2 * EM:].rearrange("(dt p) e -> p dt e", p=P))
            wqkres = consts.tile([P, DT, 4, P], bf16)  # Q/K weights, resident
            nc.sync.dma_start(
                wqkres, wqkvT[:, 0:2 * EM]
                .rearrange("(dt p) (e c) -> p dt e c", p=P, e=4))
            owres = consts.tile([P, DT, D], bf16)    # out-proj weights, resident
            nc.sync.dma_start(
                owres, outwT[:, :].rearrange("(dp p) e -> p dp e", p=P))

            cc2_in = [dram.tile([P, T], bf16, name=f"cc2i{h}") for h in range(2)]
            cc2_out = [dram.tile([4 * P, T], bf16, name=f"cc2o{h}")
                       for h in range(2)]
            stats_d = dram.tile([2, 4, TOK], f32)    # [0]=rstd, [1]=negm rows

            # q/k/v for my 4 heads over all tokens, then banded attention
            with tc.tile_pool(name="p_qkv", bufs=1) as p_qkv:
                xfull = p_qkv.tile([P, DT, 4, TOK], bf16, tag="xfull")
                nc.sync.dma_start(
                    xfull,
                    xTf[:, :].rearrange("(dt p) (tg t) -> p dt tg t", p=P, t=TOK))
                # qk[:, 0:2, tg, :] = Q^T e-tiles, qk[:, 2:4, tg, :] = K^T
                qk = p_qkv.tile([P, 4, 4, TOK], bf16, tag="qk")
                vb = p_qkv.tile([P, KT, 4, HD + 1], bf16, tag="vb")
                ones_kh = p_qkv.tile([P, KT * 4], f32, tag="ones_kh")
                nc.vector.memset(ones_kh, 1.0)
                nc.vector.tensor_copy(
                    vb[:, :, :, HD],
                    ones_kh.rearrange("p (k h) -> p k h", h=4))

                with tc.tile_pool(name="qkvc", bufs=1) as qkvc:
                    # ---- LN1 stats for ALL tokens, locally ----
                    negm = qkvc.tile([1, 4, TOK], bf16)
                    with (
                        tc.tile_pool(name="st", bufs=1) as st,
                        tc.tile_pool(name="stps", bufs=2, space="PSUM") as stps,
                    ):
                        for tg in range(4):
                            ps_s = stps.tile([1, TOK], f32, tag="ps_s",
                                             name="ps_s")
                            ps_q = stps.tile([1, TOK], f32, tag="ps_q",
                                             name="ps_q")
                            sqs = []
                            for dt in range(DT):
                                sq = st.tile([P, TOK], bf16, tag=f"sq{dt}",
                                             name=f"sq{dt}")
                                nc.vector.tensor_mul(sq, xfull[:, dt, tg, :],
                                                     xfull[:, dt, tg, :])
                                sqs.append(sq)
                            for dt in range(DT):
                                nc.tensor.matmul(ps_s, ones, xfull[:, dt, tg, :],
                                                 start=(dt == 0),
                                                 stop=(dt == DT - 1))
                            for dt in range(DT):
                                nc.tensor.matmul(ps_q, ones, sqs[dt],
                                                 start=(dt == 0),
                                                 stop=(dt == DT - 1))
                            mean = st.tile([1, TOK], f32, tag="mean")
                            nc.vector.tensor_scalar_mul(mean, ps_s, 1.0 / D)
                            var = st.tile([1, TOK], f32, tag="var")
                            nc.vector.tensor_scalar_mul(var, ps_q, 1.0 / D)
                            msq = st.tile([1, TOK], f32, tag="msq")
                            nc.vector.tensor_mul(msq, mean, mean)
                            nc.vector.tensor_sub(var, var, msq)
                            lnv = st.tile([1, TOK], f32, tag="lnv")
                            nc.scalar.activation(out=lnv, in_=var, func=ACT.Ln,
                                                 bias=epst, scale=1.0)
                            rstd1 = st.tile([1, TOK], f32, tag="rstd1")
                            nc.scalar.activation(out=rstd1, in_=lnv,
                                                 func=ACT.Exp, scale=-0.5)
                            nc.sync.dma_start(stats_d[0, tg, :], rstd1)
                            nc.vector.tensor_scalar_mul(negm[:, tg, :], mean,
                                                        -1.0)
                    rT = qkvc.tile([P, KT], f32)          # rstd, token-partition
                    nc.sync.dma_start(
                        rT, stats_d.opt()[0].rearrange("(tt p) -> p tt", p=P))
                    rbc = qkvc.tile([P, 4, TOK], f32)     # rstd bcast over parts
                    nc.gpsimd.dma_start(
                        rbc, stats_d.opt()[0:1].partition_broadcast(P))

                    # ---- QKV matmuls (all weights + x resident in SBUF) ----
                    with (
                        tc.tile_pool(name="qkps", bufs=3, space="PSUM") as qkps,
                        tc.tile_pool(name="vps", bufs=3, space="PSUM") as vps,
                    ):
                        for tg in range(4):
                            for et in range(4):
                                ps = qkps.tile([P, TOK], f32, tag="ps",
                                               name="ps")
                                for dt in range(DT):
                                    nc.tensor.matmul(
                                        ps, wqkres[:, dt, et, :],
                                        xfull[:, dt, tg, :],
                                        start=(dt == 0), stop=False)
                                nc.tensor.matmul(
                                    ps, uqk_s[:, et * P:(et + 1) * P],
                                    negm[:, tg, :], start=False, stop=True)
                                t = epil.tile([P, TOK], f32, tag="ep")
                                nc.vector.tensor_mul(t, ps, rbc[:, tg, :])
                                nc.vector.tensor_scalar_add(
                                    qk[:, et, tg, :], t,
                                    scalar1=cqk_t[:, et:et + 1])
                            for sub in range(4):
                                tt = tg * 4 + sub
                                psv = vps.tile([P, EM], f32, tag="psv",
                                               name="psv")
                                for dt in range(DT):
                                    nc.tensor.matmul(
                                        psv,
                                        xfull[:, dt, tg, sub * P:(sub + 1) * P],
                                        wvres[:, dt, :],
                                        start=(dt == 0), stop=False)
                                nc.tensor.matmul(
                                    psv,
                                    negm[:, tg, sub * P:(sub + 1) * P], uv_s,
                                    start=False, stop=True)
                                nc.vector.scalar_tensor_tensor(
                                    vb[:, tt, :, 0:HD],
                                    psv.rearrange("p (h e) -> p h e", h=4),
                                    rT[:, tt:tt + 1],
                                    cvb.rearrange("p (h e) -> p h e", h=4),
                                    op0=ALU.mult, op1=ALU.add)

                # ======= attention (4 local heads, banded causal+WIN) =======
                with (
                    tc.tile_pool(name="ptp", bufs=2) as ptp,
                    tc.tile_pool(name="denp", bufs=2) as denp,
                    tc.tile_pool(name="otp", bufs=2) as otp,
                    tc.tile_pool(name="pss", bufs=2, space="PSUM") as pss,
                    tc.tile_pool(name="pso", bufs=2, space="PSUM") as pso,
                ):
                    for hp in range(2):      # head pairs: local 2hp, 2hp+1
                        if hp == 1:
                            # gather head-pair 0's output under pair 1's compute
                            nc.gpsimd.collective_compute(
                                "AllGather", ALU.bypass, replica_groups=groups,
                                ins=[cc2_in[0].opt()], outs=[cc2_out[0].opt()])
                        for qg in range(4):
                            ktg = min(KT, 4 * (qg + 1) + 1)
                            ps_o = [pso.tile([P, TOK], f32, tag=f"ps_o{hh}",
                                             name=f"ps_o{hh}") for hh in range(2)]
                            for kk in range(ktg):
                                qi_min = max(0, kk - 4 * qg - 1)
                                q0 = qi_min * P
                                ps_s = [pss.tile([P, TOK], f32, tag=f"ps_s{hh}",
                                                 name=f"ps_s{hh}")
                                        for hh in range(2)]
                                pt = [ptp.tile([P, TOK], bf16, tag=f"pt{hh}",
                                               name=f"pt{hh}") for hh in range(2)]
                                for hh in range(2):
                                    nc.tensor.matmul(
                                        ps_s[hh][:, q0:TOK],
                                        qk[hh * HD:(hh + 1) * HD, 2 + hp,
                                           kk // 4, (kk % 4) * P:(kk % 4 + 1) * P],
                                        qk[hh * HD:(hh + 1) * HD, hp, qg, q0:TOK],
                                        start=True, stop=True)
                                    for qi in range(qi_min, 4):
                                        rel = kk - (4 * qg + qi)
                                        if rel == 0 or rel == 1:
                                            m = mA if rel == 0 else mB
                                            nc.vector.tensor_add(
                                                ps_s[hh][:, qi * P:(qi + 1) * P],
                                                ps_s[hh][:, qi * P:(qi + 1) * P],
                                                m)
                                    nc.scalar.activation(
                                        out=pt[hh][:, q0:TOK],
                                        in_=ps_s[hh][:, q0:TOK], func=ACT.Exp)
                                    nc.tensor.matmul(
                                        ps_o[hh][0:HD + 1, q0:TOK],
                                        vb[:, kk, 2 * hp + hh, :],
                                        pt[hh][:, q0:TOK],
                                        start=(kk == 0), stop=(kk == ktg - 1),
                                        skip_group_check=True)
                            for hh in range(2):
                                hl = 2 * hp + hh
                                # denominator: copy to SBUF, PE-broadcast to 64
                                # partitions, reciprocal, multiply.
                                den = denp.tile([1, TOK], f32r, tag="den")
                                nc.vector.tensor_copy(den, ps_o[hh][HD:HD + 1, :])
                                dps = pss.tile([HD, TOK], f32, tag="ps_s0",
                                               name="dps")
                                nc.tensor.matmul(dps, ones64r, den,
                                                 start=True, stop=True)
                                rec = denp.tile([HD, TOK], f32, tag="rec")
                                nc.vector.reciprocal(rec, dps)
                                ot = otp.tile([HD, TOK], bf16, tag="ot")
                                nc.vector.tensor_mul(ot, ps_o[hh][0:HD, :], rec)
                                nc.sync.dma_start(
                                    cc2_in[hp][hh * HD:(hh + 1) * HD,
                                               qg * TOK:(qg + 1) * TOK], ot)

            nc.gpsimd.collective_compute(
                "AllGather", ALU.bypass, replica_groups=groups,
                ins=[cc2_in[1].opt()], outs=[cc2_out[1].opt()])

            # =========== out-proj + residual + LN2 + FFN (my tokens) =======
            with tc.tile_pool(name="p_late", bufs=1) as p_late:
                # prefetch ALL of w1 while the AllGather runs
                w1res = p_late.tile([P, DT, F], bf16, tag="w1res")
                nc.sync.dma_start(
                    w1res, w1T[:, :].rearrange("(dt p) e -> p dt e", p=P))

                toff = (nc.partition_id() % 4) * TOK
                x1 = p_late.tile([P, DT, TOK], f32, tag="x1")
                with (
                    tc.tile_pool(name="otl", bufs=2) as otl,
                    tc.tile_pool(name="mmps2", bufs=3, space="PSUM") as mmps2,
                ):
                    OT = otl.tile([P, DT, TOK], bf16, tag="OT")
                    for dpt in range(DT):
                        s, hpx = dpt // 2, dpt % 2
                        nc.sync.dma_start(
                            OT[:, dpt, :],
                            cc2_out[hpx].opt()[s * P:(s + 1) * P,
                                               bass.ds(toff, TOK)])
                    for dt in range(DT):
                        xm = otl.tile([P, TOK], f32, tag="xm", name="xm")
                        nc.sync.dma_start(xm, xTm[dt * P:(dt + 1) * P, :])
                        ps = mmps2.tile([P, TOK], f32, tag="ps")
                        for dpt in range(DT):
                            nc.tensor.matmul(
                                ps, owres[:, dpt, dt * P:(dt + 1) * P],
                                OT[:, dpt, :],
                                start=(dpt == 0), stop=(dpt == DT - 1))
                        nc.vector.tensor_scalar_add(
                            x1[:, dt, :], ps, scalar1=outb_t[:, dt:dt + 1])
                        nc.vector.tensor_add(x1[:, dt, :], x1[:, dt, :], xm)

                # LN2
                h2 = p_late.tile([P, DT, TOK], bf16, tag="h2")
                with (
                    tc.tile_pool(name="lnp", bufs=1) as lnp,
                    tc.tile_pool(name="lnps", bufs=1, space="PSUM") as lnps,
                ):
                    ps_s = lnps.tile([1, TOK], f32, tag="ps_s")
                    ps_q = lnps.tile([1, TOK], f32, tag="ps_q")
                    x1bs, sq2s = [], []
                    for half in range(2):
                        x1b = lnp.tile([P, 4, TOK], bf16, tag=f"x1b{half}",
                                       name=f"x1b{half}")
                        sq = lnp.tile([P, 4, TOK], bf16, tag=f"sq2{half}",
                                      name=f"sq2{half}")
                        for k in range(4):
                            dt = half * 4 + k
                            nc.vector.tensor_copy(x1b[:, k, :], x1[:, dt, :])
                            nc.vector.tensor_mul(sq[:, k, :], x1[:, dt, :],
                                                 x1[:, dt, :])
                        x1bs.append(x1b)
                        sq2s.append(sq)
                    for dt in range(DT):
                        nc.tensor.matmul(ps_s, ones, x1bs[dt // 4][:, dt % 4, :],
                                         start=(dt == 0), stop=(dt == DT - 1))
                    for dt in range(DT):
                        nc.tensor.matmul(ps_q, ones, sq2s[dt // 4][:, dt % 4, :],
                                         start=(dt == 0), stop=(dt == DT - 1))
                    mean = lnp.tile([1, TOK], f32, tag="mean")
                    nc.vector.tensor_scalar_mul(mean, ps_s, 1.0 / D)
                    var = lnp.tile([1, TOK], f32, tag="var")
                    nc.vector.tensor_scalar_mul(var, ps_q, 1.0 / D)
                    lnv = lnp.tile([1, TOK], f32, tag="lnv")
                    nc.vector.tensor_mul(lnv, mean, mean)
                    nc.vector.tensor_sub(var, var, lnv)
                    nc.scalar.activation(out=lnv, in_=var, func=ACT.Ln,
                                         bias=epst, scale=1.0)
                    rstd = lnp.tile([1, TOK], f32, tag="rstd")
                    nc.scalar.activation(out=rstd, in_=lnv, func=ACT.Exp,
                                         scale=-0.5)
                    nc.vector.tensor_mul(mean, mean, rstd)   # mean <- mean*rstd
                    st2d = dram.tile([2, TOK], f32)
                    nc.sync.dma_start(st2d[0:1, :], rstd)
                    nc.sync.dma_start(st2d[1:2, :], mean)
                    rstd_b = lnp.tile([P, TOK], f32, tag="rstd_b")
                    nc.gpsimd.dma_start(
                        rstd_b, st2d.opt()[0:1, :].partition_broadcast(P))
                    mr_b = lnp.tile([P, TOK], f32, tag="mr_b")
                    nc.gpsimd.dma_start(
                        mr_b, st2d.opt()[1:2, :].partition_broadcast(P))
                    for dt in range(DT):
                        t = lnp.tile([P, TOK], f32, tag="lnt")
                        nc.vector.tensor_mul(t, x1[:, dt, :], rstd_b)
                        nc.vector.tensor_sub(t, t, mr_b)
                        nc.vector.tensor_scalar(
                            h2[:, dt, :], t,
                            scalar1=g2t[:, dt:dt + 1], scalar2=bt2t[:, dt:dt + 1],
                            op0=ALU.mult, op1=ALU.add)

                # FFN1 from resident w1 (no weight DMAs)
                gt_ = p_late.tile([P, FT, TOK], bf16, tag="gT")
                with tc.tile_pool(name="mmps3", bufs=3, space="PSUM") as mmps3:
                    for ft in range(FT):
                        ps = mmps3.tile([P, TOK], f32, tag="ps", name="ps")
                        for dt in range(DT):
                            nc.tensor.matmul(
                                ps, w1res[:, dt, ft * P:(ft + 1) * P],
                                h2[:, dt, :],
                                start=(dt == 0), stop=(dt == DT - 1))
                        nc.scalar.activation(
                            out=gt_[:, ft, :], in_=ps, func=ACT.Gelu,
                            bias=b1_t[:, ft:ft + 1], scale=1.0)
                    # FFN2: chunks of 8 f-tiles x 4 d-outputs, run-length 8
                    ffn2cm = tc.tile_pool(name="ffn2ps", bufs=1, space="PSUM")
                    ffn2ps = ffn2cm.__enter__()
                    for dg in range(2):
                        ps4 = [ffn2ps.tile([P, TOK], f32, tag=f"psf{j}",
                                           name=f"psf{j}") for j in range(4)]
                        for fc in range(4):
                            w8 = wpool.tile([P, 8, 4, P], bf16, tag="w8")
                            nc.sync.dma_start(
                                w8, w2T[fc * 8 * P:(fc + 1) * 8 * P,
                                        dg * 4 * P:(dg + 1) * 4 * P]
                                .rearrange("(ft p) (e c) -> p ft e c", p=P, e=4))
                            for j in range(4):
                                for f8 in range(8):
                                    nc.tensor.matmul(
                                        ps4[j], w8[:, f8, j, :],
                                        gt_[:, fc * 8 + f8, :],
                                        start=(fc == 0 and f8 == 0),
                                        stop=(fc == 3 and f8 == 7))
                        for j in range(4):
                            dt = dg * 4 + j
                            o = epil.tile([P, TOK], f32, tag="ep")
                            nc.vector.tensor_scalar_add(
                                o, ps4[j], scalar1=b2_t[:, dt:dt + 1])
                            nc.vector.tensor_add(o, o, x1[:, dt, :])
                            nc.sync.dma_start(yT[dt * P:(dt + 1) * P, :], o)
                    ffn2cm.__exit__(None, None, None)

    return nc


@functools.lru_cache(maxsize=1)
def _build():
    import concourse.tile as tile
    import concourse.mybir as mybir
    from concourse import bacc

    nc = bacc.Bacc("TRN2")
    _emit(nc, tile, mybir)
    nc.compile()
    return nc


def _host_prep(inputs):
    """Transpose weights/x; fold LN1 gamma/beta + 1/sqrt(hd) into QKV."""
    import ml_dtypes

    bf = ml_dtypes.bfloat16
    x = np.asarray(inputs["x"], dtype=np.float32)
    ipw = np.asarray(inputs["in_proj_w"], dtype=np.float32)
    ipb = np.asarray(inputs["in_proj_b"], dtype=np.float32)
    g1 = np.asarray(inputs["ln1_g"], dtype=np.float32)
    bt1 = np.asarray(inputs["ln1_b"], dtype=np.float32)
    sc = 1.0 / np.sqrt(np.float32(HD))
    wsc = np.concatenate([ipw[:D] * sc, ipw[D:]], axis=0)   # q-scale folded
    bsc = np.concatenate([ipb[:D] * sc, ipb[D:]], axis=0)
    wfold = wsc * g1[None, :]                               # LN1 gamma folded
    ufold = wfold.sum(axis=1)                               # [3D]
    cfold = wsc @ bt1 + bsc                                 # LN1 beta folded

    shared = {
        "outwT": np.ascontiguousarray(
            np.asarray(inputs["out_w"], np.float32).T.astype(bf)),
        "w1T": np.ascontiguousarray(
            np.asarray(inputs["w1"], np.float32).T.astype(bf)),
        "w2T": np.ascontiguousarray(
            np.asarray(inputs["w2"], np.float32).T.astype(bf)),
        "outb": np.ascontiguousarray(np.asarray(inputs["out_b"], np.float32)),
        "b1": np.ascontiguousarray(np.asarray(inputs["b1"], np.float32)),
        "b2": np.ascontiguousarray(np.asarray(inputs["b2"], np.float32)),
        "ln2g": np.ascontiguousarray(np.asarray(inputs["ln2_g"], np.float32)),
        "ln2b": np.ascontiguousarray(np.asarray(inputs["ln2_b"], np.float32)),
    }
    ii, jj = np.arange(P)[:, None], np.arange(P)[None, :]
    shared["maskA"] = np.where(ii <= jj + WIN, 0.0, NEG).astype(np.float32)
    shared["maskB"] = np.where(ii <= jj - (P - WIN), 0.0, NEG).astype(np.float32)

    xT = [np.ascontiguousarray(x[b].T) for b in range(B)]
    xTb = [np.ascontiguousarray(t.astype(bf)) for t in xT]

    in_maps = []
    for c in range(NCORES):
        b, r = c // 4, c % 4
        qs = slice(EM * r, EM * (r + 1))            # my q rows in [0, D)
        ks = slice(D + EM * r, D + EM * (r + 1))
        vs = slice(2 * D + EM * r, 2 * D + EM * (r + 1))
        m = dict(shared)
        m["xTf"] = xTb[b]
        m["xTm"] = np.ascontiguousarray(xT[b][:, TOK * r: TOK * (r + 1)])
        m["wqkvT"] = np.ascontiguousarray(
            np.concatenate([wfold[qs], wfold[ks], wfold[vs]], axis=0).T.astype(bf))
        m["uqk"] = np.ascontiguousarray(
            np.concatenate([ufold[qs], ufold[ks]]).reshape(1, -1).astype(bf))
        m["uv"] = np.ascontiguousarray(ufold[vs].reshape(1, -1).astype(bf))
        m["cqk"] = np.ascontiguousarray(
            np.concatenate([cfold[qs], cfold[ks]]))
        m["cv"] = np.ascontiguousarray(cfold[vs].reshape(1, -1))
        in_maps.append(m)
    return in_maps


def kernel(**inputs) -> np.ndarray:
    from concourse.bass_utils import run_bass_kernel_spmd

    nc = _build()
    in_maps = _host_prep(inputs)
    res = run_bass_kernel_spmd(nc, in_maps, core_ids=list(range(NCORES)),
                               trace=False)
    y = np.empty((B, T, D), dtype=np.float32)
    for c in range(NCORES):
        b, r = c // 4, c % 4
        y[b, TOK * r: TOK * (r + 1), :] = res.results[c]["yT"].T
    return y


# revision 26
# speedup vs baseline: 1.1687x; 1.0359x over previous
"""Trainium2 Bass kernel for a pre-norm look-ahead-window transformer block.

Problem: y = x + FFN(LN2(x + Attn(LN1(x)))) with banded-causal attention
(key j visible to query i iff j <= i + WIN), B=2, T=2048, D=1024, H=16.

Distribution over 8 NeuronCores (one SPMD program, uniform across cores):
  core c: batch b = c//4, rank r = c%4 in the 4-core batch group.
  - residual / out-proj / FFN are token-sharded (512 tokens per core).
  - QKV + attention are head-sharded (4 heads, all 2048 tokens), which keeps
    the banded-attention loop structure identical on every core. LN1 is
    folded into the QKV weights on the host (W' = W*g per column, rank-1
    mean correction via a K=1 matmul, per-token rstd scale in the epilogue);
    LN1 stats for all tokens are computed locally on every core (cheaper
    than a collective's latency floor).
  - ONE 4-rank AllGather moves the attention output O^T (1 MB bf16); all
    FFN1 weights are prefetched into SBUF underneath it. The post-gather
    token slice is selected with a partition-id-driven dynamic DMA offset.

Everything runs in a transposed [feature, token] layout so matmuls contract
over SBUF partitions; the host pre-transposes x / weights and transposes
the per-core [D, 512] output shards back. Matmul inputs are bf16 (full PE
rate); residuals and PSUM accumulation stay fp32.
"""

import functools

import numpy as np

B, T, D, H, HD, WIN, F = 2, 2048, 1024, 16, 64, 10, 4096
P = 128
NCORES = 8
TOK = 512          # tokens per core for the token-sharded phases
DT = D // P        # 8 d-tiles
FT = F // P        # 32 f-tiles
KT = T // P        # 16 key tiles
EM = 2 * P         # 256 e-rows per core per projection (4 heads x 64)
EPS = 1e-5
NEG = -30000.0     # additive mask; exp(NEG) == 0 exactly on ACT


def _emit(nc, tile, mybir):
    import concourse.bass as bass

    f32 = mybir.dt.float32
    f32r = mybir.dt.float32r
    bf16 = mybir.dt.bfloat16
    ALU = mybir.AluOpType
    ACT = mybir.ActivationFunctionType

    # ---- kernel I/O ----
    xTf = nc.dram_tensor("xTf", [D, T], bf16, kind="ExternalInput")
    xTm = nc.dram_tensor("xTm", [D, TOK], f32, kind="ExternalInput")
    wqkvT = nc.dram_tensor("wqkvT", [D, 3 * EM], bf16, kind="ExternalInput")
    uqk = nc.dram_tensor("uqk", [1, 2 * EM], bf16, kind="ExternalInput")
    uv = nc.dram_tensor("uv", [1, EM], bf16, kind="ExternalInput")
    cqk = nc.dram_tensor("cqk", [2 * EM], f32, kind="ExternalInput")
    cv = nc.dram_tensor("cv", [1, EM], f32, kind="ExternalInput")
    outwT = nc.dram_tensor("outwT", [D, D], bf16, kind="ExternalInput")
    w1T = nc.dram_tensor("w1T", [D, F], bf16, kind="ExternalInput")
    w2T = nc.dram_tensor("w2T", [F, D], bf16, kind="ExternalInput")
    outb = nc.dram_tensor("outb", [D], f32, kind="ExternalInput")
    b1 = nc.dram_tensor("b1", [F], f32, kind="ExternalInput")
    b2 = nc.dram_tensor("b2", [D], f32, kind="ExternalInput")
    ln2g = nc.dram_tensor("ln2g", [D], f32, kind="ExternalInput")
    ln2b = nc.dram_tensor("ln2b", [D], f32, kind="ExternalInput")
    maskA = nc.dram_tensor("maskA", [P, P], f32, kind="ExternalInput")
    maskB = nc.dram_tensor("maskB", [P, P], f32, kind="ExternalInput")
    yT = nc.dram_tensor("yT", [D, TOK], f32, kind="ExternalOutput")

    groups = [[0, 1, 2, 3], [4, 5, 6, 7]]

    with tile.TileContext(nc) as tc:
        from contextlib import ExitStack

        with ExitStack() as ctx:
            consts = ctx.enter_context(tc.tile_pool(name="consts", bufs=1))
            epil = ctx.enter_context(tc.tile_pool(name="epil", bufs=4))
            wpool = ctx.enter_context(tc.tile_pool(name="wpool", bufs=2))
            dram = ctx.enter_context(tc.tile_pool(name="dram", bufs=1, space="DRAM"))

            # ---- constants ----
            ones_f = consts.tile([P, 1], f32)
            nc.vector.memset(ones_f, 1.0)
            ones = consts.tile([P, 1], bf16)
            nc.vector.tensor_copy(ones, ones_f)
            ones64r = consts.tile([1, HD], f32r)
            nc.vector.tensor_copy(ones64r, ones_f[0:1, 0:1].to_broadcast((1, HD)))
            ones128r = consts.tile([1, P], f32r)
            nc.vector.tensor_copy(ones128r, ones_f[0:1, 0:1].to_broadcast((1, P)))
            mA = consts.tile([P, P], f32)
            nc.sync.dma_start(mA, maskA[:, :])
            mB = consts.tile([P, P], f32)
            nc.sync.dma_start(mB, maskB[:, :])

            def load_pp(dramvec, n):  # [n*P] -> [P, n] per-partition layout
                t = consts.tile([P, n], f32, tag=f"pp_{dramvec.name}")
                nc.sync.dma_start(t, dramvec[:].rearrange("(n p) -> p n", p=P))
                return t

            cqk_t = load_pp(cqk, 4)
            outb_t = load_pp(outb, DT)
            b1_t = load_pp(b1, FT)
            b2_t = load_pp(b2, DT)
            g2t = load_pp(ln2g, DT)
            bt2t = load_pp(ln2b, DT)
            epst = consts.tile([1, 1], f32)
            nc.vector.memset(epst, EPS)
            uqk_s = consts.tile([1, 2 * EM], bf16)
            nc.sync.dma_start(uqk_s, uqk[:, :])
            uv_s = consts.tile([1, EM], bf16)
            nc.sync.dma_start(uv_s, uv[:, :])
            cvb = consts.tile([P, EM], f32)
            nc.gpsimd.dma_start(cvb, cv[:, :].partition_broadcast(P))
            wvres = consts.tile([P, DT, EM], bf16)   # V weights, resident
            nc.sync.dma_start(
                wvres, wqkvT[:, 2 * EM:].rearrange("(dt p) e -> p dt e", p=P))
            wqkres = consts.tile([P, DT, 4, P], bf16)  # Q/K weights, resident
            nc.sync.dma_start(
                wqkres, wqkvT[:, 0:2 * EM]
                .rearrange("(dt p) (e c) -> p dt e c", p=P, e=4))
            owres = consts.tile([P, DT, D], bf16)    # out-proj weights, resident
            nc.sync.dma_start(
                owres, outwT[:, :].rearrange("(dp p) e -> p dp e", p=P))

            cc2_in = [dram.tile([P, T], bf16, name=f"cc2i{h}") for h in range(2)]
            cc2_out = [dram.tile([4 * P, T], bf16, name=f"cc2o{h}")
                       for h in range(2)]
            stats_d = dram.tile([2, 4, TOK], f32)    # [0]=rstd, [1]=negm rows

            # q/k/v for my 4 heads over all tokens, then banded attention
            with tc.tile_pool(name="p_qkv", bufs=1) as p_qkv:
                xfull = p_qkv.tile([P, DT, 4, TOK], bf16, tag="xfull")
                nc.sync.dma_start(
                    xfull,
                    xTf[:, :].rearrange("(dt p) (tg t) -> p dt tg t", p=P, t=TOK))
                # qk[:, 0:2, tg, :] = Q^T e-tiles, qk[:, 2:4, tg, :] = K^T
                qk = p_qkv.tile([P, 4, 4, TOK], bf16, tag="qk")
                vb = p_qkv.tile([P, KT, 4, HD + 1], bf16, tag="vb")
                ones_kh = p_qkv.tile([P, KT * 4], f32, tag="ones_kh")
                nc.vector.memset(ones_kh, 1.0)
                nc.vector.tensor_copy(
                    vb[:, :, :, HD],
                    ones_kh.rearrange("p (k h) -> p k h", h=4))

                with tc.tile_pool(name="qkvc", bufs=1) as qkvc:
                    # ---- LN1 stats for ALL tokens, locally ----
                    negm = qkvc.tile([1, 4, TOK], bf16)
                    with (
                        tc.tile_pool(name="st", bufs=1) as st,
                        tc.tile_pool(name="stps", bufs=2, space="PSUM") as stps,
                    ):
                        for tg in range(4):
                            ps_s = stps.tile([1, TOK], f32, tag="ps_s",
                                             name="ps_s")
                            ps_q = stps.tile([1, TOK], f32, tag="ps_q",
                                             name="ps_q")
                            sqs = []
                            for dt in range(DT):
                                sq = st.tile([P, TOK], bf16, tag=f"sq{dt}",
                                             name=f"sq{dt}")
                                nc.vector.tensor_mul(sq, xfull[:, dt, tg, :],
                                                     xfull[:, dt, tg, :])
                                sqs.append(sq)
                            for dt in range(DT):
                                nc.tensor.matmul(ps_s, ones, xfull[:, dt, tg, :],
                                                 start=(dt == 0),
                                                 stop=(dt == DT - 1))
                            for dt in range(DT):
                                nc.tensor.matmul(ps_q, ones, sqs[dt],
                                                 start=(dt == 0),
                                                 stop=(dt == DT - 1))
                            mean = st.tile([1, TOK], f32, tag="mean")
                            nc.vector.tensor_scalar_mul(mean, ps_s, 1.0 / D)
                            var = st.tile([1, TOK], f32, tag="var")
                            nc.vector.tensor_scalar_mul(var, ps_q, 1.0 / D)
                            msq = st.tile([1, TOK], f32, tag="msq")
                            nc.vector.tensor_mul(msq, mean, mean)
                            nc.vector.tensor_sub(var, var, msq)
                            lnv = st.tile([1, TOK], f32, tag="lnv")
                            nc.scalar.activation(out=lnv, in_=var, func=ACT.Ln,
                                                 bias=epst, scale=1.0)
                            rstd1 = st.tile([1, TOK], f32, tag="rstd1")
                            nc.scalar.activation(out=rstd1, in_=lnv,
                                                 func=ACT.Exp, scale=-0.5)
                            nc.sync.dma_start(stats_d[0, tg, :], rstd1)
                            nc.vector.tensor_scalar_mul(negm[:, tg, :], mean,
                                                        -1.0)
                    rT = qkvc.tile([P, KT], f32)          # rstd, token-partition
                    nc.sync.dma_start(
                        rT, stats_d.opt()[0].rearrange("(tt p) -> p tt", p=P))
                    rbc = qkvc.tile([P, 4, TOK], f32)     # rstd bcast over parts
                    nc.gpsimd.dma_start(
                        rbc, stats_d.opt()[0:1].partition_broadcast(P))

                    # ---- QKV matmuls (all weights + x resident in SBUF) ----
                    with (
                        tc.tile_pool(name="qkps", bufs=3, space="PSUM") as qkps,
                        tc.tile_pool(name="vps", bufs=3, space="PSUM") as vps,
                    ):
                        for tg in range(4):
                            for et in range(4):
                                ps = qkps.tile([P, TOK], f32, tag="ps",
                                               name="ps")
                                for dt in range(DT):
                                    nc.tensor.matmul(
                                        ps, wqkres[:, dt, et, :],
                                        xfull[:, dt, tg, :],
                                        start=(dt == 0), stop=False)
                                nc.tensor.matmul(
                                    ps, uqk_s[:, et * P:(et + 1) * P],
                                    negm[:, tg, :], start=False, stop=True)
                                t = epil.tile([P, TOK], f32, tag="ep")
                                nc.vector.tensor_mul(t, ps, rbc[:, tg, :])
                                nc.vector.tensor_scalar_add(
                                    qk[:, et, tg, :], t,
                                    scalar1=cqk_t[:, et:et + 1])
                            for sub in range(4):
                                tt = tg * 4 + sub
                                psv = vps.tile([P, EM], f32, tag="psv",
                                               name="psv")
                                for dt in range(DT):
                                    nc.tensor.matmul(
                                        psv,
                                        xfull[:, dt, tg, sub * P:(sub + 1) * P],
                                        wvres[:, dt, :],
                                        start=(dt == 0), stop=False)
                                nc.tensor.matmul(
                                    psv,
                                    negm[:, tg, sub * P:(sub + 1) * P], uv_s,
                                    start=False, stop=True)
                                nc.vector.scalar_tensor_tensor(
                                    vb[:, tt, :, 0:HD],
                                    psv.rearrange("p (h e) -> p h e", h=4),
                                    rT[:, tt:tt + 1],
                                    cvb.rearrange("p (h e) -> p h e", h=4),
                                    op0=ALU.mult, op1=ALU.add)

                # ======= attention (4 local heads, banded causal+WIN) =======
                with (
                    tc.tile_pool(name="ptp", bufs=2) as ptp,
                    tc.tile_pool(name="denp", bufs=2) as denp,
                    tc.tile_pool(name="otp", bufs=2) as otp,
                    tc.tile_pool(name="pss", bufs=2, space="PSUM") as pss,
                    tc.tile_pool(name="pso", bufs=1, space="PSUM") as pso,
                    tc.tile_pool(name="dpsp", bufs=2, space="PSUM") as dpsp,
                ):
                    for hp in range(2):      # head pairs: local 2hp, 2hp+1
                        if hp == 1:
                            # gather head-pair 0's output under pair 1's compute
                            nc.gpsimd.collective_compute(
                                "AllGather", ALU.bypass, replica_groups=groups,
                                ins=[cc2_in[0].opt()], outs=[cc2_out[0].opt()])
                        for qg in range(4):
                            ktg = min(KT, 4 * (qg + 1) + 1)
                            ps_o = [pso.tile([P, TOK], f32, tag=f"ps_o{hh}",
                                             name=f"ps_o{hh}") for hh in range(2)]
                            for kk in range(ktg):
                                qi_min = max(0, kk - 4 * qg - 1)
                                q0 = qi_min * P
                                ps2 = pss.tile([P, 2, TOK], f32, tag="ps2",
                                               name="ps2")
                                pt2 = ptp.tile([P, 2, TOK], bf16, tag="pt2",
                                               name="pt2")
                                for hh in range(2):
                                    nc.tensor.matmul(
                                        ps2[:, hh, q0:TOK],
                                        qk[hh * HD:(hh + 1) * HD, 2 + hp,
                                           kk // 4, (kk % 4) * P:(kk % 4 + 1) * P],
                                        qk[hh * HD:(hh + 1) * HD, hp, qg, q0:TOK],
                                        start=True, stop=True)
                                for qi in range(qi_min, 4):
                                    rel = kk - (4 * qg + qi)
                                    if rel == 0 or rel == 1:
                                        m = mA if rel == 0 else mB
                                        nc.vector.tensor_add(
                                            ps2[:, :, qi * P:(qi + 1) * P],
                                            ps2[:, :, qi * P:(qi + 1) * P],
                                            m[:, None, :].to_broadcast(
                                                (P, 2, P)))
                                nc.scalar.activation(
                                    out=pt2[:, :, q0:TOK],
                                    in_=ps2[:, :, q0:TOK], func=ACT.Exp)
                                for hh in range(2):
                                    nc.tensor.matmul(
                                        ps_o[hh][0:HD + 1, q0:TOK],
                                        vb[:, kk, 2 * hp + hh, :],
                                        pt2[:, hh, q0:TOK],
                                        start=(kk == 0), stop=(kk == ktg - 1),
                                        skip_group_check=True)
                            for hh in range(2):
                                hl = 2 * hp + hh
                                # denominator: copy to SBUF, PE-broadcast to 64
                                # partitions, reciprocal, multiply.
                                den = denp.tile([1, TOK], f32r, tag="den")
                                nc.vector.tensor_copy(den, ps_o[hh][HD:HD + 1, :])
                                dps = dpsp.tile([HD, TOK], f32, tag="dps",
                                               name="dps")
                                nc.tensor.matmul(dps, ones64r, den,
                                                 start=True, stop=True)
                                rec = denp.tile([HD, TOK], f32, tag="rec")
                                nc.vector.reciprocal(rec, dps)
                                ot = otp.tile([HD, TOK], bf16, tag="ot")
                                nc.vector.tensor_mul(ot, ps_o[hh][0:HD, :], rec)
                                nc.sync.dma_start(
                                    cc2_in[hp][hh * HD:(hh + 1) * HD,
                                               qg * TOK:(qg + 1) * TOK], ot)

            nc.gpsimd.collective_compute(
                "AllGather", ALU.bypass, replica_groups=groups,
                ins=[cc2_in[1].opt()], outs=[cc2_out[1].opt()])

            # =========== out-proj + residual + LN2 + FFN (my tokens) =======
            with tc.tile_pool(name="p_late", bufs=1) as p_late:
                # prefetch w1 (first half) while the AllGather runs
                w1res = p_late.tile([P, DT, F], bf16, tag="w1res")
                nc.sync.dma_start(
                    w1res[:, :, 0:F // 2],
                    w1T[:, 0:F // 2].rearrange("(dt p) e -> p dt e", p=P))

                toff = (nc.partition_id() % 4) * TOK
                x1 = p_late.tile([P, DT, TOK], f32, tag="x1")
                with (
                    tc.tile_pool(name="otl", bufs=2) as otl,
                    tc.tile_pool(name="mmps2", bufs=3, space="PSUM") as mmps2,
                ):
                    OT = otl.tile([P, DT, TOK], bf16, tag="OT")
                    for dpt in range(DT):
                        s, hpx = dpt // 2, dpt % 2
                        nc.sync.dma_start(
                            OT[:, dpt, :],
                            cc2_out[hpx].opt()[s * P:(s + 1) * P,
                                               bass.ds(toff, TOK)])
                    for dt in range(DT):
                        xm = otl.tile([P, TOK], f32, tag="xm", name="xm")
                        nc.sync.dma_start(xm, xTm[dt * P:(dt + 1) * P, :])
                        ps = mmps2.tile([P, TOK], f32, tag="ps")
                        for dpt in range(DT):
                            nc.tensor.matmul(
                                ps, owres[:, dpt, dt * P:(dt + 1) * P],
                                OT[:, dpt, :],
                                start=(dpt == 0), stop=(dpt == DT - 1))
                        nc.vector.tensor_scalar_add(
                            x1[:, dt, :], ps, scalar1=outb_t[:, dt:dt + 1])
                        nc.vector.tensor_add(x1[:, dt, :], x1[:, dt, :], xm)

                # second half of the w1 prefetch (after the OT loads)
                nc.sync.dma_start(
                    w1res[:, :, F // 2:],
                    w1T[:, F // 2:].rearrange("(dt p) e -> p dt e", p=P))

                # LN2
                h2 = p_late.tile([P, DT, TOK], bf16, tag="h2")
                with (
                    tc.tile_pool(name="lnp", bufs=1) as lnp,
                    tc.tile_pool(name="lnps", bufs=1, space="PSUM") as lnps,
                ):
                    ps_s = lnps.tile([1, TOK], f32, tag="ps_s")
                    ps_q = lnps.tile([1, TOK], f32, tag="ps_q")
                    x1bs, sq2s = [], []
                    for half in range(2):
                        x1b = lnp.tile([P, 4, TOK], bf16, tag=f"x1b{half}",
                                       name=f"x1b{half}")
                        sq = lnp.tile([P, 4, TOK], bf16, tag=f"sq2{half}",
                                      name=f"sq2{half}")
                        for k in range(4):
                            dt = half * 4 + k
                            nc.vector.tensor_copy(x1b[:, k, :], x1[:, dt, :])
                            nc.vector.tensor_mul(sq[:, k, :], x1[:, dt, :],
                                                 x1[:, dt, :])
                        x1bs.append(x1b)
                        sq2s.append(sq)
                    for dt in range(DT):
                        nc.tensor.matmul(ps_s, ones, x1bs[dt // 4][:, dt % 4, :],
                                         start=(dt == 0), stop=(dt == DT - 1))
                    for dt in range(DT):
                        nc.tensor.matmul(ps_q, ones, sq2s[dt // 4][:, dt % 4, :],
                                         start=(dt == 0), stop=(dt == DT - 1))
                    mean = lnp.tile([1, TOK], f32, tag="mean")
                    nc.vector.tensor_scalar_mul(mean, ps_s, 1.0 / D)
                    var = lnp.tile([1, TOK], f32, tag="var")
                    nc.vector.tensor_scalar_mul(var, ps_q, 1.0 / D)
                    lnv = lnp.tile([1, TOK], f32, tag="lnv")
                    nc.vector.tensor_mul(lnv, mean, mean)
                    nc.vector.tensor_sub(var, var, lnv)
                    nc.scalar.activation(out=lnv, in_=var, func=ACT.Ln,
                                         bias=epst, scale=1.0)
                    rstd = lnp.tile([1, TOK], f32, tag="rstd")
                    nc.scalar.activation(out=rstd, in_=lnv, func=ACT.Exp,
                                         scale=-0.5)
                    nc.vector.tensor_mul(mean, mean, rstd)   # mean <- mean*rstd
                    rsr = lnp.tile([1, TOK], f32r, tag="rsr")
                    nc.vector.tensor_copy(rsr, rstd)
                    mrr = lnp.tile([1, TOK], f32r, tag="mrr")
                    nc.vector.tensor_copy(mrr, mean)
                    rstd_b = lnps.tile([P, TOK], f32, tag="rstd_b")
                    nc.tensor.matmul(rstd_b, ones128r, rsr, start=True, stop=True)
                    mr_b = lnps.tile([P, TOK], f32, tag="mr_b")
                    nc.tensor.matmul(mr_b, ones128r, mrr, start=True, stop=True)
                    for dt in range(DT):
                        t = lnp.tile([P, TOK], f32, tag="lnt")
                        nc.vector.tensor_mul(t, x1[:, dt, :], rstd_b)
                        nc.vector.tensor_sub(t, t, mr_b)
                        nc.vector.tensor_scalar(
                            h2[:, dt, :], t,
                            scalar1=g2t[:, dt:dt + 1], scalar2=bt2t[:, dt:dt + 1],
                            op0=ALU.mult, op1=ALU.add)

                # FFN1 from resident w1 (no weight DMAs)
                gt_ = p_late.tile([P, FT, TOK], bf16, tag="gT")
                with tc.tile_pool(name="mmps3", bufs=3, space="PSUM") as mmps3:
                    for ft in range(FT):
                        ps = mmps3.tile([P, TOK], f32, tag="ps", name="ps")
                        for dt in range(DT):
                            nc.tensor.matmul(
                                ps, w1res[:, dt, ft * P:(ft + 1) * P],
                                h2[:, dt, :],
                                start=(dt == 0), stop=(dt == DT - 1))
                        nc.scalar.activation(
                            out=gt_[:, ft, :], in_=ps, func=ACT.Gelu,
                            bias=b1_t[:, ft:ft + 1], scale=1.0)
                    # FFN2: chunks of 8 f-tiles x 4 d-outputs, run-length 8
                    ffn2cm = tc.tile_pool(name="ffn2ps", bufs=1, space="PSUM")
                    ffn2ps = ffn2cm.__enter__()
                    for dg in range(2):
                        ps4 = [ffn2ps.tile([P, TOK], f32, tag=f"psf{j}",
                                           name=f"psf{j}") for j in range(4)]
                        for fc in range(4):
                            w8 = wpool.tile([P, 8, 4, P], bf16, tag="w8")
                            nc.sync.dma_start(
                                w8, w2T[fc * 8 * P:(fc + 1) * 8 * P,
                                        dg * 4 * P:(dg + 1) * 4 * P]
                                .rearrange("(ft p) (e c) -> p ft e c", p=P, e=4))
                            for j in range(4):
                                for f8 in range(8):
                                    nc.tensor.matmul(
                                        ps4[j], w8[:, f8, j, :],
                                        gt_[:, fc * 8 + f8, :],
                                        start=(fc == 0 and f8 == 0),
                                        stop=(fc == 3 and f8 == 7))
                        for j in range(4):
                            dt = dg * 4 + j
                            o = epil.tile([P, TOK], f32, tag="ep")
                            nc.vector.tensor_scalar_add(
                                o, ps4[j], scalar1=b2_t[:, dt:dt + 1])
                            nc.vector.tensor_add(o, o, x1[:, dt, :])
                            nc.sync.dma_start(yT[dt * P:(dt + 1) * P, :], o)
                    ffn2cm.__exit__(None, None, None)

    return nc


@functools.lru_cache(maxsize=1)
def _build():
    import concourse.tile as tile
    import concourse.mybir as mybir
    from concourse import bacc

    nc = bacc.Bacc("TRN2")
    _emit(nc, tile, mybir)
    nc.compile()
    return nc


def _host_prep(inputs):
    """Transpose weights/x; fold LN1 gamma/beta + 1/sqrt(hd) into QKV."""
    import ml_dtypes

    bf = ml_dtypes.bfloat16
    x = np.asarray(inputs["x"], dtype=np.float32)
    ipw = np.asarray(inputs["in_proj_w"], dtype=np.float32)
    ipb = np.asarray(inputs["in_proj_b"], dtype=np.float32)
    g1 = np.asarray(inputs["ln1_g"], dtype=np.float32)
    bt1 = np.asarray(inputs["ln1_b"], dtype=np.float32)
    sc = 1.0 / np.sqrt(np.float32(HD))
    wsc = np.concatenate([ipw[:D] * sc, ipw[D:]], axis=0)   # q-scale folded
    bsc = np.concatenate([ipb[:D] * sc, ipb[D:]], axis=0)
    wfold = wsc * g1[None, :]                               # LN1 gamma folded
    ufold = wfold.sum(axis=1)                               # [3D]
    cfold = wsc @ bt1 + bsc                                 # LN1 beta folded

    shared = {
        "outwT": np.ascontiguousarray(
            np.asarray(inputs["out_w"], np.float32).T.astype(bf)),
        "w1T": np.ascontiguousarray(
            np.asarray(inputs["w1"], np.float32).T.astype(bf)),
        "w2T": np.ascontiguousarray(
            np.asarray(inputs["w2"], np.float32).T.astype(bf)),
        "outb": np.ascontiguousarray(np.asarray(inputs["out_b"], np.float32)),
        "b1": np.ascontiguousarray(np.asarray(inputs["b1"], np.float32)),
        "b2": np.ascontiguousarray(np.asarray(inputs["b2"], np.float32)),
        "ln2g": np.ascontiguousarray(np.asarray(inputs["ln2_g"], np.float32)),
        "ln2b": np.ascontiguousarray(np.asarray(inputs["ln2_b"], np.float32)),
    }
    ii, jj = np.arange(P)[:, None], np.arange(P)[None, :]
    shared["maskA"] = np.where(ii <= jj + WIN, 0.0, NEG).astype(np.float32)
    shared["maskB"] = np.where(ii <= jj - (P - WIN), 0.0, NEG).astype(np.float32)

    xT = [np.ascontiguousarray(x[b].T) for b in range(B)]
    xTb = [np.ascontiguousarray(t.astype(bf)) for t in xT]

    in_maps = []
    for c in range(NCORES):
        b, r = c // 4, c % 4
        qs = slice(EM * r, EM * (r + 1))            # my q rows in [0, D)
        ks = slice(D + EM * r, D + EM * (r + 1))
        vs = slice(2 * D + EM * r, 2 * D + EM * (r + 1))
        m = dict(shared)
        m["xTf"] = xTb[b]
        m["xTm"] = np.ascontiguousarray(xT[b][:, TOK * r: TOK * (r + 1)])
        m["wqkvT"] = np.ascontiguousarray(
            np.concatenate([wfold[qs], wfold[ks], wfold[vs]], axis=0).T.astype(bf))
        m["uqk"] = np.ascontiguousarray(
            np.concatenate([ufold[qs], ufold[ks]]).reshape(1, -1).astype(bf))
        m["uv"] = np.ascontiguousarray(ufold[vs].reshape(1, -1).astype(bf))
        m["cqk"] = np.ascontiguousarray(
            np.concatenate([cfold[qs], cfold[ks]]))
        m["cv"] = np.ascontiguousarray(cfold[vs].reshape(1, -1))
        in_maps.append(m)
    return in_maps


def kernel(**inputs) -> np.ndarray:
    from concourse.bass_utils import run_bass_kernel_spmd

    nc = _build()
    in_maps = _host_prep(inputs)
    res = run_bass_kernel_spmd(nc, in_maps, core_ids=list(range(NCORES)),
                               trace=False)
    y = np.empty((B, T, D), dtype=np.float32)
    for c in range(NCORES):
        b, r = c // 4, c % 4
        y[b, TOK * r: TOK * (r + 1), :] = res.results[c]["yT"].T
    return y
